# revision 12
# baseline (speedup 1.0000x reference)
"""DCRNN (2x GCNConv + GRU-over-nodes + Linear) on 8 Trainium2 cores.

Strategy (v2)
-------------
* Adjacency is stored as EXACT small-integer edge counts (A+I) in fp8e3
  (E3M4); the D^-1/2 normalization is factored out: host prescales x
  rows by dinv, the device prescales XW2 rows (per-partition scalar) and
  output columns (broadcast dinv row).  Mixed-dtype matmul (fp16
  stationary x fp8 moving) is exact on HW, and fp8 halves adjacency HBM
  traffic vs fp16.
* GCN1 is computed as (A @ x) @ W1 (x is only 64 features wide), so the
  big A-pass runs with M=64: one matmul per K-tile instead of two.
* K-enumeration: 90 tiles of 128 rows: 80 "global" (core, j) tiles with
  each core's own tiles zeroed, plus that core's 10 tiles duplicated at
  the end.  Both GCN layers stream one adjacency layout (interleaved in
  groups of 5 tiles -> 6.5KB DMA descriptors); GCN2 starts on the own
  tiles (local XW2 shard + adjacency groups retained in SBUF from GCN1)
  while the XW2 AllGather is still in flight.
* The XW2 AllGather uses a tiled layout ([128 part, 10*256] per core,
  5KB descriptors) and is split in two (tiles j<5 / j>=5) so GCN2's
  even groups only wait on the first half.
* GRU over the 10000-node sequence: 8 Jacobi fixed-point sweeps; gates
  from the previous sweep's h (ping-pong buffers), then the recurrence
  h_t = z_t h_{t-1} + (1-z_t) n_t applied EXACTLY with the DVE affine
  scan, chunk-chained (per-partition initial state) so scans overlap
  the next chunk's gate matmuls.  Pointwise work is spread over
  scalar/vector/gpsimd.  A 64-row halo makes cores independent.
"""

import numpy as np

NUM_NODES = 10000
IN_FEAT = 64
HID = 256
OUT = 3
CORES = 8
ROWS = NUM_NODES // CORES          # 1250
HALO = 64
L = ROWS + HALO                    # 1314 local sequence length
SWEEPS = 8
KP = 128
NJ = 10                            # local node tiles per core (10*128 >= 1250)
NT = CORES * NJ                    # 80 global K-tiles
NT2 = NT + NJ                      # + own tiles duplicated at the end
GK = 5                             # K-tiles per interleave group
NG2 = NT2 // GK                    # 18 groups; own tiles = groups 16,17

_CACHE = {}


def _chunks(total, step=512):
    return [(c, min(c + step, total)) for c in range(0, total, step)]


def build_program():
    import concourse.mybir as mybir
    import concourse.tile as tile
    from concourse import bacc

    f16 = mybir.dt.float16
    f8 = mybir.dt.float8e3
    f32 = mybir.dt.float32
    AF = mybir.ActivationFunctionType
    ALU = mybir.AluOpType

    nc = bacc.Bacc("TRN2", num_devices=CORES)

    # ---- inputs ----
    adj_d = nc.dram_tensor("adj", [NG2 * KP, GK * L], f8, kind="ExternalInput")
    xe_d = nc.dram_tensor("xe", [KP, NT2 * IN_FEAT], f16, kind="ExternalInput")
    w1_d = nc.dram_tensor("w1", [IN_FEAT, HID], f16, kind="ExternalInput")
    w2_d = nc.dram_tensor("w2", [HID, HID], f16, kind="ExternalInput")
    wiht_d = nc.dram_tensor("wiht", [HID, 3 * HID], f16, kind="ExternalInput")
    whht_d = nc.dram_tensor("whht", [HID, 3 * HID], f16, kind="ExternalInput")
    fcwt_d = nc.dram_tensor("fcwt", [HID, OUT], f16, kind="ExternalInput")
    ident_d = nc.dram_tensor("ident", [KP, KP], f16, kind="ExternalInput")
    dr_d = nc.dram_tensor("dr", [KP, L], f16, kind="ExternalInput")
    dv_d = nc.dram_tensor("dv", [KP, NJ], f32, kind="ExternalInput")
    b1c_d = nc.dram_tensor("b1c", [KP, 2], f32, kind="ExternalInput")
    b2c_d = nc.dram_tensor("b2c", [KP, 2], f32, kind="ExternalInput")
    gib_d = nc.dram_tensor("gib", [KP, 6], f32, kind="ExternalInput")
    bhn_d = nc.dram_tensor("bhn", [KP, 2], f32, kind="ExternalInput")
    fcb_d = nc.dram_tensor("fcb", [KP, 1], f32, kind="ExternalInput")
    patch_d = nc.dram_tensor("patch", [KP, 12], f32, kind="ExternalInput")
    out_d = nc.dram_tensor("out_t", [OUT, ROWS], f32, kind="ExternalOutput")

    ch1 = _chunks(ROWS)            # GCN1 output cols (own rows only)
    ch2 = _chunks(L)               # GCN2 / GRU cols (with halo)

    with tile.TileContext(nc) as tc:
        with (
            tc.tile_pool(name="const", bufs=1) as cpool,
            tc.tile_pool(name="big", bufs=1) as big,
            tc.tile_pool(name="own", bufs=1) as ownp,
            tc.tile_pool(name="tmp", bufs=4) as tpool,
            tc.tile_pool(name="psxw", bufs=2, space="PSUM") as psxw,
            tc.tile_pool(name="dram", bufs=1, space="DRAM") as dpool,
        ):
            # ---- load constants ----
            xe_cm = tc.tile_pool(name="xep", bufs=1)
            xep = xe_cm.__enter__()
            xe_sb = xep.tile([KP, NT2 * IN_FEAT], f16)
            w1_sb = cpool.tile([IN_FEAT, HID], f16)
            w2_sb = cpool.tile([KP, 2, HID], f16)
            wiht_sb = cpool.tile([KP, 2, 3 * HID], f16)
            whht_sb = cpool.tile([KP, 2, 3 * HID], f16)
            fcwt_sb = cpool.tile([KP, 2, OUT], f16)
            ident_sb = cpool.tile([KP, KP], f16)
            dr_sb = cpool.tile([KP, L], f16)
            dv_sb = cpool.tile([KP, NJ], f32)
            b1c_sb = cpool.tile([KP, 2], f32)
            b2c_sb = cpool.tile([KP, 2], f32)
            gib_sb = cpool.tile([KP, 6], f32)
            bhn_sb = cpool.tile([KP, 2], f32)
            fcb_sb = cpool.tile([KP, 1], f32)
            patch_sb = cpool.tile([KP, 12], f32)

            # adjacency groups + xe stream on sync/scalar; everything else
            # (needed later) loads via the otherwise-idle gpsimd/vector queues
            nc.sync.dma_start(ident_sb[:], ident_d[:])
            nc.gpsimd.dma_start(w1_sb[:], w1_d[:])
            hx = NT2 * IN_FEAT // 2
            nc.scalar.dma_start(xe_sb[:, 0:hx], xe_d[:, 0:hx])
            nc.scalar.dma_start(xe_sb[:, hx:], xe_d[:, hx:])
            nc.gpsimd.dma_start(dr_sb[:], dr_d[:])
            nc.gpsimd.dma_start(dv_sb[:], dv_d[:])
            for k in range(2):
                nc.gpsimd.dma_start(w2_sb[:, k, :], w2_d[k * KP:(k + 1) * KP, :])
                nc.gpsimd.dma_start(wiht_sb[:, k, :], wiht_d[k * KP:(k + 1) * KP, :])
                nc.gpsimd.dma_start(whht_sb[:, k, :], whht_d[k * KP:(k + 1) * KP, :])
                nc.gpsimd.dma_start(fcwt_sb[:, k, :], fcwt_d[k * KP:(k + 1) * KP, :])
            nc.gpsimd.dma_start(b1c_sb[:], b1c_d[:])
            nc.gpsimd.dma_start(b2c_sb[:], b2c_d[:])
            nc.gpsimd.dma_start(gib_sb[:], gib_d[:])
            nc.gpsimd.dma_start(bhn_sb[:], bhn_d[:])
            nc.gpsimd.dma_start(fcb_sb[:], fcb_d[:])
            nc.gpsimd.dma_start(patch_sb[:], patch_d[:])

            psG_cm = tc.tile_pool(name="psG", bufs=1, space="PSUM")
            psG = psG_cm.__enter__()

            # tiny AllGather to absorb the first-collective ncfw setup cost
            ccw_in = dpool.tile([CORES, 64], f16)
            ccw_out = dpool.tile([CORES * CORES, 64], f16, addr_space="Shared")
            nc.sync.dma_start(ccw_in[0:8, :], ident_sb[0:8, 0:64])
            nc.gpsimd.collective_compute(
                "AllGather", mybir.AluOpType.bypass,
                replica_groups=[list(range(CORES))],
                ins=[ccw_in.opt()], outs=[ccw_out.opt()])

            # PE warm-up burst so the HAM clock-gate opens before GCN1
            for i in range(40):
                psd = psxw.tile([KP, 512], f32, tag="xwps", name=f"warm_{i}")
                nc.tensor.matmul(psd[:, :KP], ident_sb[:], ident_sb[:],
                                 start=True, stop=True)

            ap_cm = tc.tile_pool(name="astream", bufs=6)
            apool = ap_cm.__enter__()

            # ---- GCN1 A-pass: AxT[f, col] = sum_n x_scaled[n, f] A[n, col] ----
            # K-loop over 18 groups; own groups (16, 17) last, retained for GCN2.
            psA = [psG.tile([KP, 512], f32, tag=f"G{ci}", name=f"psA_{ci}")
                   for ci in range(3)]
            own_at = {}
            for g in range(NG2):
                if g >= NG2 - 2:
                    at = ownp.tile([KP, GK * L], f8, name=f"own_{g}")
                    own_at[g] = at
                else:
                    at = apool.tile([KP, GK * L], f8, tag="a")
                eng = nc.sync if g % 2 == 0 else nc.scalar
                eng.dma_start(at[:], adj_d[g * KP:(g + 1) * KP, :])
                for j5 in range(GK):
                    t = g * GK + j5
                    for ci, (c0, c1) in enumerate(ch1):
                        nc.tensor.matmul(
                            psA[ci][:IN_FEAT, :c1 - c0],
                            xe_sb[:, t * IN_FEAT:(t + 1) * IN_FEAT],
                            at[:, j5 * L + HALO + c0:j5 * L + HALO + c1],
                            start=(t == 0), stop=(t == NT2 - 1))

            # ---- h1T = relu(W1^T @ (dinv_col * AxT) + b1), own cols only ----
            ax_sb = big.tile([IN_FEAT, ROWS], f16)
            for ci, (c0, c1) in enumerate(ch1):
                nc.vector.tensor_mul(ax_sb[:, c0:c1], psA[ci][:IN_FEAT, :c1 - c0],
                                     dr_sb[0:IN_FEAT, HALO + c0:HALO + c1])
            h1t_sb = big.tile([KP, 2, NJ * KP], f16)
            for mm in range(2):
                nc.vector.memset(h1t_sb[:, mm, ROWS:], 0.0)
            for ci, (c0, c1) in enumerate(ch1):
                for mm in range(2):
                    ps = psxw.tile([KP, 512], f32, tag="xwps")
                    nc.tensor.matmul(ps[:, :c1 - c0],
                                     w1_sb[:, mm * KP:(mm + 1) * KP],
                                     ax_sb[:, c0:c1], start=True, stop=True)
                    nc.scalar.activation(h1t_sb[:, mm, c0:c1], ps[:, :c1 - c0],
                                         AF.Relu, bias=b1c_sb[:, mm:mm + 1])

            # ---- XW2 shard (natural layout), row-scaled by dinv; AllGather ----
            xw2l_sb = cpool.tile([KP, NJ, HID], f16)
            bounce_a = dpool.tile([KP, GK * HID], f16)
            bounce_b = dpool.tile([KP, GK * HID], f16)
            gath_a = dpool.tile([CORES * KP, GK * HID], f16, addr_space="Shared")
            gath_b = dpool.tile([CORES * KP, GK * HID], f16, addr_space="Shared")
            for j in range(NJ):
                ps = psxw.tile([KP, 512], f32, tag="xwps")
                for k in range(2):
                    nc.tensor.matmul(ps[:, :HID],
                                     h1t_sb[:, k, j * KP:(j + 1) * KP],
                                     w2_sb[:, k, :],
                                     start=(k == 0), stop=(k == 1))
                nc.scalar.activation(xw2l_sb[:, j, :], ps[:, :HID], AF.Copy,
                                     scale=dv_sb[:, j:j + 1])
                if j == GK - 1:
                    nc.sync.dma_start(bounce_a[:], xw2l_sb[:, 0:GK, :])
                    nc.gpsimd.collective_compute(
                        "AllGather", mybir.AluOpType.bypass,
                        replica_groups=[list(range(CORES))],
                        ins=[bounce_a.opt()], outs=[gath_a.opt()])
            nc.sync.dma_start(bounce_b[:], xw2l_sb[:, GK:NJ, :])
            nc.gpsimd.collective_compute(
                "AllGather", mybir.AluOpType.bypass,
                replica_groups=[list(range(CORES))],
                ins=[bounce_b.opt()], outs=[gath_b.opt()])
            xw2g_sb = big.tile([KP, CORES, NJ * HID], f16)
            for c in range(CORES):
                eng = nc.sync if c % 2 == 0 else nc.gpsimd
                eng.dma_start(xw2g_sb[:, c, 0:GK * HID],
                              gath_a[c * KP:(c + 1) * KP, :])
            for c in range(CORES):
                eng = nc.scalar if c % 2 == 0 else nc.gpsimd
                eng.dma_start(xw2g_sb[:, c, GK * HID:NJ * HID],
                              gath_b[c * KP:(c + 1) * KP, :])

            # ---- GCN2 over the extended (halo) strip ----
            # K-order: own duplicated tiles first (local XW2 + retained
            # adjacency -> runs during the AllGather), then even groups
            # (gather A), then odd groups (gather B).
            ps2 = [[psG.tile([KP, 512], f32, tag=f"G{mm * 3 + ci}",
                             name=f"ps2_{mm}_{ci}")
                    for ci in range(3)] for mm in range(2)]

            def gcn2_mm(at, j5, lhs_of, ti):
                for mm in range(2):
                    lhsT = lhs_of(mm)
                    for ci, (c0, c1) in enumerate(ch2):
                        nc.tensor.matmul(
                            ps2[mm][ci][:, :c1 - c0], lhsT,
                            at[:, j5 * L + c0:j5 * L + c1],
                            start=(ti == 0), stop=(ti == NT2 - 1))

            ti = 0
            for g in (NG2 - 2, NG2 - 1):
                at = own_at[g]
                for j5 in range(GK):
                    j = (g - (NG2 - 2)) * GK + j5
                    gcn2_mm(at, j5,
                            lambda mm, j=j: xw2l_sb[:, j, mm * KP:(mm + 1) * KP],
                            ti)
                    ti += 1
            for g in [x for x in range(NT // NJ * 2) if x % 2 == 0] + \
                    [x for x in range(NT // NJ * 2) if x % 2 == 1]:
                at = apool.tile([KP, GK * L], f8, tag="a")
                eng = nc.sync if g % 2 == 0 else nc.scalar
                eng.dma_start(at[:], adj_d[g * KP:(g + 1) * KP, :])
                cc = g // 2
                for j5 in range(GK):
                    j = (g % 2) * GK + j5
                    gcn2_mm(at, j5,
                            lambda mm, cc=cc, j=j: xw2g_sb[
                                :, cc, j * HID + mm * KP:j * HID + (mm + 1) * KP],
                            ti)
                    ti += 1

            # h2 = relu(dinv_col * agg + b2)
            h2t_sb = big.tile([KP, 2, L], f16)
            for mm in range(2):
                for ci, (c0, c1) in enumerate(ch2):
                    tt = tpool.tile([KP, 512], f16, tag="h2tmp")
                    nc.vector.tensor_mul(tt[:, :c1 - c0],
                                         ps2[mm][ci][:, :c1 - c0],
                                         dr_sb[:, c0:c1])
                    nc.scalar.activation(h2t_sb[:, mm, c0:c1], tt[:, :c1 - c0],
                                         AF.Relu, bias=b2c_sb[:, mm:mm + 1])

            ap_cm.__exit__(None, None, None)

            # ---- GI = W_ih @ h2T + (b_ih [+ b_hh for r,z]) ----
            gi_sb = big.tile([KP, 6, L], f16)
            for ci, (c0, c1) in enumerate(ch2):
                cw = c1 - c0
                psg = [psG.tile([KP, 512], f32, tag=f"G{m}", name=f"psgi_{m}")
                       for m in range(6)]
                for m in range(6):
                    for k in range(2):
                        nc.tensor.matmul(psg[m][:, :cw],
                                         wiht_sb[:, k, m * KP:(m + 1) * KP],
                                         h2t_sb[:, k, c0:c1],
                                         start=(k == 0), stop=(k == 1))
                for m in range(6):
                    if m % 2 == 0:
                        nc.scalar.activation(gi_sb[:, m, c0:c1], psg[m][:, :cw],
                                             AF.Identity, bias=gib_sb[:, m:m + 1])
                    else:
                        nc.vector.tensor_scalar_add(gi_sb[:, m, c0:c1],
                                                    psg[m][:, :cw],
                                                    gib_sb[:, m:m + 1])
            # per-core GI patch on the first HALO cols (core 0 kills its pads)
            for m in range(6):
                nc.vector.tensor_scalar(gi_sb[:, m, :HALO], gi_sb[:, m, :HALO],
                                        patch_sb[:, m:m + 1],
                                        patch_sb[:, 6 + m:7 + m],
                                        ALU.mult, ALU.add)

            # ---- GRU fixed-point sweeps (Jacobi, ping-pong h buffers) ----
            hsh = [big.tile([KP, 2, L + 1], f16, name=f"hsh{i}") for i in range(2)]
            for i in range(2):
                for mm in range(2):
                    nc.vector.memset(hsh[i][:, mm, :], 0.0)
            for s in range(SWEEPS):
                hr = hsh[s % 2]
                hw = hsh[1 - s % 2]
                z_sb = big.tile([KP, 2, L], f16, tag="Z")
                b_sb = big.tile([KP, 2, L], f16, tag="B")
                for ci, (c0, c1) in enumerate(ch2):
                    cw = c1 - c0
                    psg = [psG.tile([KP, 512], f32, tag=f"G{m}",
                                    name=f"psu_{s}_{m}") for m in range(6)]
                    # u_rz = GI (identity matmul) + W_hh_rz @ h_prev
                    for m in range(4):
                        nc.tensor.matmul(psg[m][:, :cw], ident_sb[:],
                                         gi_sb[:, m, c0:c1],
                                         start=True, stop=False)
                    for m in range(6):
                        for k in range(2):
                            nc.tensor.matmul(psg[m][:, :cw],
                                             whht_sb[:, k, m * KP:(m + 1) * KP],
                                             hr[:, k, c0:c1],
                                             start=(m >= 4 and k == 0),
                                             stop=(k == 1))
                    for mm in range(2):
                        r_t = tpool.tile([KP, 512], f16, tag="r")
                        t_t = tpool.tile([KP, 512], f16, tag="t")
                        un_t = tpool.tile([KP, 512], f16, tag="un")
                        n_t = tpool.tile([KP, 512], f16, tag="n")
                        nc.scalar.activation(r_t[:, :cw], psg[mm][:, :cw],
                                             AF.Sigmoid)
                        nc.scalar.activation(z_sb[:, mm, c0:c1],
                                             psg[2 + mm][:, :cw], AF.Sigmoid)
                        # t = (gh_n + b_hh_n) * r; mm=0 in one DVE op off
                        # PSUM, mm=1 split scalar-add + gpsimd-mult to keep
                        # the vector engine (scans live there) unsaturated
                        if mm == 0:
                            nc.vector.scalar_tensor_tensor(
                                t_t[:, :cw], psg[4 + mm][:, :cw],
                                bhn_sb[:, mm:mm + 1], r_t[:, :cw],
                                ALU.add, ALU.mult)
                        else:
                            g_t = tpool.tile([KP, 512], f16, tag="g")
                            nc.scalar.activation(g_t[:, :cw],
                                                 psg[4 + mm][:, :cw],
                                                 AF.Identity,
                                                 bias=bhn_sb[:, mm:mm + 1])
                            nc.gpsimd.tensor_mul(t_t[:, :cw], g_t[:, :cw],
                                                 r_t[:, :cw])
                        nc.gpsimd.tensor_add(un_t[:, :cw], t_t[:, :cw],
                                             gi_sb[:, 4 + mm, c0:c1])
                        nc.scalar.activation(n_t[:, :cw], un_t[:, :cw], AF.Tanh)
                        # b = (z-1)*n; scan uses op1=subtract so
                        # h = z*h_prev - b = z*h_prev + (1-z)*n
                        nc.vector.scalar_tensor_tensor(
                            b_sb[:, mm, c0:c1], z_sb[:, mm, c0:c1], 1.0,
                            n_t[:, :cw], ALU.subtract, ALU.mult)
                    # chunk-chained exact scans; overlap next chunk's gates
                    for mm in range(2):
                        nc.vector.tensor_tensor_scan(
                            hw[:, mm, c0 + 1:c1 + 1], z_sb[:, mm, c0:c1],
                            b_sb[:, mm, c0:c1], hw[:, mm, c0:c0 + 1],
                            ALU.mult, ALU.subtract)

            # ---- final Linear on the real rows (skip halo) ----
            hfin = hsh[SWEEPS % 2]
            out_sb = cpool.tile([4, ROWS], f32)
            for c0, c1 in ch1:
                cw = c1 - c0
                psf = psxw.tile([KP, 512], f32, tag="xwps")
                for k in range(2):
                    nc.tensor.matmul(psf[:OUT, :cw], fcwt_sb[:, k, :],
                                     hfin[:, k, HALO + 1 + c0:HALO + 1 + c1],
                                     start=(k == 0), stop=(k == 1))
                nc.scalar.activation(out_sb[:OUT, c0:c1], psf[:OUT, :cw],
                                     AF.Identity, bias=fcb_sb[:OUT, :])
            nc.sync.dma_start(out_d[:], out_sb[:OUT, :])

            psG_cm.__exit__(None, None, None)
            xe_cm.__exit__(None, None, None)

    nc.compile()
    return nc


def host_prepare(inputs):
    """Build the per-core input maps from the full problem inputs."""
    import ml_dtypes

    f8 = ml_dtypes.float8_e3m4
    x = np.asarray(inputs["x"], np.float32)
    ei = np.asarray(inputs["edge_index"])
    W1 = np.asarray(inputs["W1"], np.float32)
    b1 = np.asarray(inputs["b1"], np.float32)
    W2 = np.asarray(inputs["W2"], np.float32)
    b2 = np.asarray(inputs["b2"], np.float32)
    W_ih = np.asarray(inputs["W_ih"], np.float32)
    W_hh = np.asarray(inputs["W_hh"], np.float32)
    b_ih = np.asarray(inputs["b_ih"], np.float32)
    b_hh = np.asarray(inputs["b_hh"], np.float32)
    fc_w = np.asarray(inputs["fc_w"], np.float32)
    fc_b = np.asarray(inputs["fc_b"], np.float32)

    N = NUM_NODES
    src, dst = ei[0].astype(np.int64), ei[1].astype(np.int64)
    deg = np.bincount(dst, minlength=N).astype(np.float64) + 1.0
    dinv = (1.0 / np.sqrt(deg)).astype(np.float32)

    # Exact integer adjacency counts (A + I), transposed view A8[s, d]
    A8 = np.zeros((N, N), np.int8)
    np.add.at(A8, (src, dst), 1)
    idx = np.arange(N)
    A8[idx, idx] += 1

    # node enumeration for the 80 global K-tiles: tile (c,j), partition p
    # -> node c*1250 + j*128 + p (invalid slots padded)
    enum = np.full(NT * KP, -1, np.int64)
    for c in range(CORES):
        for j in range(NJ):
            base = c * ROWS + j * KP
            cnt = min(KP, ROWS - j * KP)
            s0 = (c * NJ + j) * KP
            enum[s0:s0 + cnt] = base + np.arange(cnt)
    valid = enum >= 0
    env = enum[valid]

    # x prescaled by dinv, laid out [128 part, tile, feat]
    xd = (x * dinv[:, None]).astype(np.float32)
    xe_g = np.zeros((NT * KP, IN_FEAT), np.float32)
    xe_g[valid] = xd[env]

    common = {
        "w1": W1.astype(np.float16),
        "w2": W2.astype(np.float16),
        "wiht": W_ih.T.astype(np.float16),
        "whht": W_hh.T.astype(np.float16),
        "fcwt": fc_w.T.astype(np.float16),
        "ident": np.eye(KP, dtype=np.float16),
        "b1c": b1.reshape(2, KP).T.astype(np.float32).copy(),
        "b2c": b2.reshape(2, KP).T.astype(np.float32).copy(),
        "gib": (b_ih + np.concatenate([b_hh[:2 * HID],
                                       np.zeros(HID, np.float32)])
                ).reshape(6, KP).T.astype(np.float32).copy(),
        "bhn": b_hh[2 * HID:].reshape(2, KP).T.astype(np.float32).copy(),
        "fcb": np.concatenate([fc_b, np.zeros(KP - OUT, np.float32)]
                              ).reshape(KP, 1),
    }

    in_maps = []
    for c in range(CORES):
        r0, r1 = c * ROWS, (c + 1) * ROWS
        lo = r0 - HALO
        # per-core strip of adjacency columns [lo, r1), rows in K-enum order
        strip = np.zeros((NT2 * KP, L), np.int8)
        if c == 0:
            strip[:NT * KP][valid, HALO:] = A8[env, 0:r1]
        else:
            strip[:NT * KP][valid, :] = A8[env, lo:r1]
        # duplicate own tiles at the end; zero them in the global block
        o0, o1 = c * NJ * KP, (c + 1) * NJ * KP
        strip[NT * KP:] = strip[o0:o1]
        strip[o0:o1] = 0
        # interleave in groups of GK tiles: row g*128+p, col j5*L+cc
        adj = np.ascontiguousarray(
            strip.reshape(NG2, GK, KP, L).transpose(0, 2, 1, 3)
        ).reshape(NG2 * KP, GK * L).astype(f8)

        xe_e = np.zeros((NT2 * KP, IN_FEAT), np.float32)
        xe_e[:NT * KP] = xe_g
        xe_e[NT * KP:] = xe_g[o0:o1]
        xe = np.ascontiguousarray(
            xe_e.reshape(NT2, KP, IN_FEAT).transpose(1, 0, 2)
        ).reshape(KP, NT2 * IN_FEAT).astype(np.float16)

        # dinv of the strip's column nodes, broadcast over partitions
        drow = np.zeros(L, np.float32)
        if c == 0:
            drow[HALO:] = dinv[0:r1]
        else:
            drow[:] = dinv[lo:r1]
        dr = np.broadcast_to(drow.astype(np.float16), (KP, L)).copy()

        # dinv per (partition, local tile) for XW2 row scaling (0 on pads)
        dv = np.zeros((KP, NJ), np.float32)
        for j in range(NJ):
            cnt = min(KP, ROWS - j * KP)
            dv[:cnt, j] = dinv[r0 + j * KP:r0 + j * KP + cnt]

        patch = np.zeros((KP, 12), np.float32)
        if c == 0:
            # mul=0; add=-60 for r,z gate tiles, 0 for n tiles -> pad cols
            # produce exactly h=0 so row 0 starts from the true h0=0.
            patch[:, 6:10] = -60.0
        else:
            patch[:, 0:6] = 1.0
        in_maps.append({**common, "adj": adj, "xe": xe, "dr": dr, "dv": dv,
                        "patch": patch})
    return in_maps


def assemble_output(results):
    outs = [r["out_t"].T for r in results]          # each [ROWS, OUT]
    full = np.concatenate(outs, axis=0).astype(np.float32)
    return full[None]                               # [1, N, OUT]


def kernel(**inputs) -> np.ndarray:
    from concourse import bass_utils

    if "nc" not in _CACHE:
        _CACHE["nc"] = build_program()
    nc = _CACHE["nc"]
    in_maps = host_prepare(inputs)
    res = bass_utils.run_bass_kernel_spmd(
        nc, in_maps, core_ids=list(range(CORES)))
    return assemble_output(res.results)


if __name__ == "__main__":
    import reference

    inputs = {k: np.asarray(v) for k, v in reference.setup_inputs().items()}
    out = kernel(**inputs)
    print("kernel out", out.shape, out.dtype)
    np.save("/root/problem/kernel_out.npy", out)


# revision 19
# speedup vs baseline: 1.0054x; 1.0054x over previous
"""DCRNN (2x GCNConv + GRU-over-nodes + Linear) on 8 Trainium2 cores.

Strategy (v2)
-------------
* Adjacency is stored as EXACT small-integer edge counts (A+I) in fp8e3
  (E3M4); the D^-1/2 normalization is factored out: host prescales x
  rows by dinv, the device prescales XW2 rows (per-partition scalar) and
  output columns (broadcast dinv row).  Mixed-dtype matmul (fp16
  stationary x fp8 moving) is exact on HW, and fp8 halves adjacency HBM
  traffic vs fp16.
* GCN1 is computed as (A @ x) @ W1 (x is only 64 features wide), so the
  big A-pass runs with M=64: one matmul per K-tile instead of two.
* K-enumeration: 90 tiles of 128 rows: 80 "global" (core, j) tiles with
  each core's own tiles zeroed, plus that core's 10 tiles duplicated at
  the end.  Both GCN layers stream one adjacency layout (interleaved in
  groups of 5 tiles -> 6.5KB DMA descriptors); GCN2 starts on the own
  tiles (local XW2 shard + adjacency groups retained in SBUF from GCN1)
  while the XW2 AllGather is still in flight.
* The XW2 AllGather uses a tiled layout ([128 part, 10*256] per core,
  5KB descriptors) and is split in two (tiles j<5 / j>=5) so GCN2's
  even groups only wait on the first half.
* GRU over the 10000-node sequence: 8 Jacobi fixed-point sweeps; gates
  from the previous sweep's h (ping-pong buffers), then the recurrence
  h_t = z_t h_{t-1} + (1-z_t) n_t applied EXACTLY with the DVE affine
  scan, chunk-chained (per-partition initial state) so scans overlap
  the next chunk's gate matmuls.  Pointwise work is spread over
  scalar/vector/gpsimd.  A 64-row halo makes cores independent.
"""

import numpy as np

NUM_NODES = 10000
IN_FEAT = 64
HID = 256
OUT = 3
CORES = 8
ROWS = NUM_NODES // CORES          # 1250
HALO = 64
L = ROWS + HALO                    # 1314 local sequence length
SWEEPS = 8
KP = 128
NJ = 10                            # local node tiles per core (10*128 >= 1250)
NT = CORES * NJ                    # 80 global K-tiles
NT2 = NT + NJ                      # + own tiles duplicated at the end
GK = 5                             # K-tiles per interleave group
NG2 = NT2 // GK                    # 18 groups; own tiles = groups 16,17

_CACHE = {}


def _chunks(total, step=512):
    return [(c, min(c + step, total)) for c in range(0, total, step)]


def build_program():
    import concourse.mybir as mybir
    import concourse.tile as tile
    from concourse import bacc

    f16 = mybir.dt.float16
    f8 = mybir.dt.float8e4
    f32 = mybir.dt.float32
    DR = mybir.MatmulPerfMode.DoubleRow
    AF = mybir.ActivationFunctionType
    ALU = mybir.AluOpType

    nc = bacc.Bacc("TRN2", num_devices=CORES)

    # ---- inputs ----
    adj_d = nc.dram_tensor("adj", [NG2 * KP, GK * L], f8, kind="ExternalInput")
    xe_d = nc.dram_tensor("xe", [KP, NT2 * IN_FEAT], f16, kind="ExternalInput")
    w1_d = nc.dram_tensor("w1", [IN_FEAT, HID], f16, kind="ExternalInput")
    w2_d = nc.dram_tensor("w2", [HID, HID], f16, kind="ExternalInput")
    wiht_d = nc.dram_tensor("wiht", [HID, 3 * HID], f16, kind="ExternalInput")
    whht_d = nc.dram_tensor("whht", [HID, 3 * HID], f16, kind="ExternalInput")
    fcwt_d = nc.dram_tensor("fcwt", [HID, OUT], f16, kind="ExternalInput")
    ident_d = nc.dram_tensor("ident", [KP, KP], f16, kind="ExternalInput")
    dr_d = nc.dram_tensor("dr", [KP, L], f16, kind="ExternalInput")
    dv_d = nc.dram_tensor("dv", [KP, NJ], f32, kind="ExternalInput")
    b1c_d = nc.dram_tensor("b1c", [KP, 2], f32, kind="ExternalInput")
    b2c_d = nc.dram_tensor("b2c", [KP, 2], f32, kind="ExternalInput")
    gib_d = nc.dram_tensor("gib", [KP, 6], f32, kind="ExternalInput")
    bhn_d = nc.dram_tensor("bhn", [KP, 2], f32, kind="ExternalInput")
    fcb_d = nc.dram_tensor("fcb", [KP, 1], f32, kind="ExternalInput")
    patch_d = nc.dram_tensor("patch", [KP, 12], f32, kind="ExternalInput")
    out_d = nc.dram_tensor("out_t", [OUT, ROWS], f32, kind="ExternalOutput")

    ch1 = _chunks(ROWS)            # GCN1 output cols (own rows only)
    ch2 = _chunks(L)               # GCN2 / GRU cols (with halo)

    with tile.TileContext(nc) as tc:
        with (
            tc.tile_pool(name="const", bufs=1) as cpool,
            tc.tile_pool(name="big", bufs=1) as big,
            tc.tile_pool(name="own", bufs=1) as ownp,
            tc.tile_pool(name="tmp", bufs=4) as tpool,
            tc.tile_pool(name="psxw", bufs=2, space="PSUM") as psxw,
            tc.tile_pool(name="dram", bufs=1, space="DRAM") as dpool,
        ):
            # ---- load constants ----
            xe_cm = tc.tile_pool(name="xep", bufs=1)
            xep = xe_cm.__enter__()
            xe_sb = xep.tile([KP, NT2 * IN_FEAT], f16)
            w1_sb = cpool.tile([IN_FEAT, HID], f16)
            w2_sb = cpool.tile([KP, 2, HID], f16)
            wiht_sb = cpool.tile([KP, 2, 3 * HID], f16)
            whht_sb = cpool.tile([KP, 2, 3 * HID], f16)
            fcwt_sb = cpool.tile([KP, 2, OUT], f16)
            ident_sb = cpool.tile([KP, KP], f16)
            dr_sb = cpool.tile([KP, L], f16)
            dv_sb = cpool.tile([KP, NJ], f32)
            b1c_sb = cpool.tile([KP, 2], f32)
            b2c_sb = cpool.tile([KP, 2], f32)
            gib_sb = cpool.tile([KP, 6], f32)
            bhn_sb = cpool.tile([KP, 2], f32)
            fcb_sb = cpool.tile([KP, 1], f32)
            patch_sb = cpool.tile([KP, 12], f32)

            # adjacency groups + xe stream on sync/scalar; everything else
            # (needed later) loads via the otherwise-idle gpsimd/vector queues
            nc.sync.dma_start(ident_sb[:], ident_d[:])
            nc.gpsimd.dma_start(w1_sb[:], w1_d[:])
            hx = NT2 * IN_FEAT // 2
            nc.scalar.dma_start(xe_sb[:, 0:hx], xe_d[:, 0:hx])
            nc.scalar.dma_start(xe_sb[:, hx:], xe_d[:, hx:])
            nc.gpsimd.dma_start(dr_sb[:], dr_d[:])
            nc.gpsimd.dma_start(dv_sb[:], dv_d[:])
            for k in range(2):
                nc.gpsimd.dma_start(w2_sb[:, k, :], w2_d[k * KP:(k + 1) * KP, :])
                nc.gpsimd.dma_start(wiht_sb[:, k, :], wiht_d[k * KP:(k + 1) * KP, :])
                nc.gpsimd.dma_start(whht_sb[:, k, :], whht_d[k * KP:(k + 1) * KP, :])
                nc.gpsimd.dma_start(fcwt_sb[:, k, :], fcwt_d[k * KP:(k + 1) * KP, :])
            nc.gpsimd.dma_start(b1c_sb[:], b1c_d[:])
            nc.gpsimd.dma_start(b2c_sb[:], b2c_d[:])
            nc.gpsimd.dma_start(gib_sb[:], gib_d[:])
            nc.gpsimd.dma_start(bhn_sb[:], bhn_d[:])
            nc.gpsimd.dma_start(fcb_sb[:], fcb_d[:])
            nc.gpsimd.dma_start(patch_sb[:], patch_d[:])

            psG_cm = tc.tile_pool(name="psG", bufs=1, space="PSUM")
            psG = psG_cm.__enter__()

            # tiny AllGather to absorb the first-collective ncfw setup cost
            ccw_in = dpool.tile([CORES, 64], f16)
            ccw_out = dpool.tile([CORES * CORES, 64], f16, addr_space="Shared")
            nc.sync.dma_start(ccw_in[0:8, :], ident_sb[0:8, 0:64])
            nc.gpsimd.collective_compute(
                "AllGather", mybir.AluOpType.bypass,
                replica_groups=[list(range(CORES))],
                ins=[ccw_in.opt()], outs=[ccw_out.opt()])

            # PE warm-up burst so the HAM clock-gate opens before GCN1
            for i in range(40):
                psd = psxw.tile([KP, 512], f32, tag="xwps", name=f"warm_{i}")
                nc.tensor.matmul(psd[:, :KP], ident_sb[:], ident_sb[:],
                                 start=True, stop=True)

            ap_cm = tc.tile_pool(name="astream", bufs=6)
            apool = ap_cm.__enter__()

            # ---- GCN1 A-pass: AxT[f, col] = sum_n x_scaled[n, f] A[n, col] ----
            # K-loop over 18 groups; own groups (16, 17) last, retained for GCN2.
            psA = [psG.tile([KP, 512], f32, tag=f"G{ci}", name=f"psA_{ci}")
                   for ci in range(3)]
            own_at = {}
            for g in range(NG2):
                if g >= NG2 - 2:
                    at = ownp.tile([KP, GK, L], f8, name=f"own_{g}")
                    own_at[g] = at
                else:
                    at = apool.tile([KP, GK, L], f8, tag="a")
                eng = nc.sync if g % 2 == 0 else nc.scalar
                eng.dma_start(at[:], adj_d[g * KP:(g + 1) * KP, :])
                for j5 in range(GK):
                    t = g * GK + j5
                    for ci, (c0, c1) in enumerate(ch1):
                        nc.tensor.matmul(
                            psA[ci][:IN_FEAT, :c1 - c0],
                            xe_sb[:, t * IN_FEAT:(t + 1) * IN_FEAT],
                            at[:, j5, HALO + c0:HALO + c1],
                            start=(t == 0), stop=(t == NT2 - 1))

            # ---- h1T = relu(W1^T @ (dinv_col * AxT) + b1), own cols only ----
            ax_sb = big.tile([IN_FEAT, ROWS], f16)
            for ci, (c0, c1) in enumerate(ch1):
                nc.vector.tensor_mul(ax_sb[:, c0:c1], psA[ci][:IN_FEAT, :c1 - c0],
                                     dr_sb[0:IN_FEAT, HALO + c0:HALO + c1])
            h1t_sb = big.tile([KP, 2, NJ * KP], f16)
            for mm in range(2):
                nc.vector.memset(h1t_sb[:, mm, ROWS:], 0.0)
            for ci, (c0, c1) in enumerate(ch1):
                for mm in range(2):
                    ps = psxw.tile([KP, 512], f32, tag="xwps")
                    nc.tensor.matmul(ps[:, :c1 - c0],
                                     w1_sb[:, mm * KP:(mm + 1) * KP],
                                     ax_sb[:, c0:c1], start=True, stop=True)
                    nc.scalar.activation(h1t_sb[:, mm, c0:c1], ps[:, :c1 - c0],
                                         AF.Relu, bias=b1c_sb[:, mm:mm + 1])

            # ---- XW2 shard (natural layout), row-scaled by dinv; AllGather.
            # fp8e4: feeds the DoubleRow GCN2 matmuls and halves gather bytes.
            xw2l_sb = cpool.tile([KP, NJ, HID], f8)
            bounce_a = dpool.tile([KP, GK * HID], f8)
            bounce_b = dpool.tile([KP, GK * HID], f8)
            gath_a = dpool.tile([CORES * KP, GK * HID], f8, addr_space="Shared")
            gath_b = dpool.tile([CORES * KP, GK * HID], f8, addr_space="Shared")
            for j in range(NJ):
                ps = psxw.tile([KP, 512], f32, tag="xwps")
                for k in range(2):
                    nc.tensor.matmul(ps[:, :HID],
                                     h1t_sb[:, k, j * KP:(j + 1) * KP],
                                     w2_sb[:, k, :],
                                     start=(k == 0), stop=(k == 1))
                nc.scalar.activation(xw2l_sb[:, j, :], ps[:, :HID], AF.Copy,
                                     scale=dv_sb[:, j:j + 1])
                if j == GK - 1:
                    nc.sync.dma_start(bounce_a[:], xw2l_sb[:, 0:GK, :])
                    nc.gpsimd.collective_compute(
                        "AllGather", mybir.AluOpType.bypass,
                        replica_groups=[list(range(CORES))],
                        ins=[bounce_a.opt()], outs=[gath_a.opt()])
            nc.sync.dma_start(bounce_b[:], xw2l_sb[:, GK:NJ, :])
            nc.gpsimd.collective_compute(
                "AllGather", mybir.AluOpType.bypass,
                replica_groups=[list(range(CORES))],
                ins=[bounce_b.opt()], outs=[gath_b.opt()])
            xw2g_sb = big.tile([KP, CORES, NJ, HID], f8)
            for c in range(CORES):
                eng = nc.sync if c % 2 == 0 else nc.gpsimd
                eng.dma_start(xw2g_sb[:, c, 0:GK, :],
                              gath_a[c * KP:(c + 1) * KP, :])
            for c in range(CORES):
                eng = nc.scalar if c % 2 == 0 else nc.gpsimd
                eng.dma_start(xw2g_sb[:, c, GK:NJ, :],
                              gath_b[c * KP:(c + 1) * KP, :])

            # ---- GCN2 over the extended (halo) strip ----
            # K-order: own duplicated tiles first (local XW2 + retained
            # adjacency -> runs during the AllGather), then even groups
            # (gather A), then odd groups (gather B).
            ps2 = [[psG.tile([KP, 512], f32, tag=f"G{mm * 3 + ci}",
                             name=f"ps2_{mm}_{ci}")
                    for ci in range(3)] for mm in range(2)]

            def gcn2_pair(at, j5, lhs_of, first, last):
                # DoubleRow: two K-tiles per matmul ([128, 2, *] operands)
                for mm in range(2):
                    lhsT = lhs_of(mm)
                    for ci, (c0, c1) in enumerate(ch2):
                        nc.tensor.matmul(
                            ps2[mm][ci][:, :c1 - c0], lhsT,
                            at[:, j5:j5 + 2, c0:c1],
                            start=first, stop=last, perf_mode=DR)

            def gcn2_one(at, j5, lhs_of, first, last):
                for mm in range(2):
                    lhsT = lhs_of(mm)
                    for ci, (c0, c1) in enumerate(ch2):
                        nc.tensor.matmul(
                            ps2[mm][ci][:, :c1 - c0], lhsT,
                            at[:, j5, c0:c1],
                            start=first, stop=last)

            def gcn2_group(at, jb, lhs_pair, lhs_one, first, last):
                # group = tiles jb..jb+4 of one core: 2 DoubleRow pairs + 1
                for j5, pair in ((0, True), (2, True), (4, False)):
                    j = jb + j5
                    if pair:
                        gcn2_pair(at, j5, lambda mm, j=j: lhs_pair(mm, j),
                                  first and j5 == 0, False)
                    else:
                        gcn2_one(at, j5, lambda mm, j=j: lhs_one(mm, j),
                                 False, last)

            own_pair = lambda mm, j: xw2l_sb[:, j:j + 2, mm * KP:(mm + 1) * KP]
            own_one = lambda mm, j: xw2l_sb[:, j, mm * KP:(mm + 1) * KP]
            for g in (NG2 - 2, NG2 - 1):
                gcn2_group(own_at[g], (g - (NG2 - 2)) * GK,
                           own_pair, own_one, g == NG2 - 2, False)
            glist = [x for x in range(NT // NJ * 2) if x % 2 == 0] + \
                    [x for x in range(NT // NJ * 2) if x % 2 == 1]
            for gi_, g in enumerate(glist):
                at = apool.tile([KP, GK, L], f8, tag="a")
                eng = nc.sync if g % 2 == 0 else nc.scalar
                eng.dma_start(at[:], adj_d[g * KP:(g + 1) * KP, :])
                cc = g // 2
                gp = lambda mm, j, cc=cc: xw2g_sb[:, cc, j:j + 2,
                                                  mm * KP:(mm + 1) * KP]
                go = lambda mm, j, cc=cc: xw2g_sb[:, cc, j,
                                                  mm * KP:(mm + 1) * KP]
                gcn2_group(at, (g % 2) * GK, gp, go,
                           False, gi_ == len(glist) - 1)

            # h2 = relu(dinv_col * agg + b2)
            h2t_sb = big.tile([KP, 2, L], f16)
            for mm in range(2):
                for ci, (c0, c1) in enumerate(ch2):
                    tt = tpool.tile([KP, 512], f16, tag="h2tmp")
                    nc.vector.tensor_mul(tt[:, :c1 - c0],
                                         ps2[mm][ci][:, :c1 - c0],
                                         dr_sb[:, c0:c1])
                    nc.scalar.activation(h2t_sb[:, mm, c0:c1], tt[:, :c1 - c0],
                                         AF.Relu, bias=b2c_sb[:, mm:mm + 1])

            ap_cm.__exit__(None, None, None)

            # ---- GI = W_ih @ h2T + (b_ih [+ b_hh for r,z]) ----
            gi_sb = big.tile([KP, 6, L], f16)
            for ci, (c0, c1) in enumerate(ch2):
                cw = c1 - c0
                psg = [psG.tile([KP, 512], f32, tag=f"G{m}", name=f"psgi_{m}")
                       for m in range(6)]
                for m in range(6):
                    for k in range(2):
                        nc.tensor.matmul(psg[m][:, :cw],
                                         wiht_sb[:, k, m * KP:(m + 1) * KP],
                                         h2t_sb[:, k, c0:c1],
                                         start=(k == 0), stop=(k == 1))
                for m in range(6):
                    if m % 2 == 0:
                        nc.scalar.activation(gi_sb[:, m, c0:c1], psg[m][:, :cw],
                                             AF.Identity, bias=gib_sb[:, m:m + 1])
                    else:
                        nc.vector.tensor_scalar_add(gi_sb[:, m, c0:c1],
                                                    psg[m][:, :cw],
                                                    gib_sb[:, m:m + 1])
            # per-core GI patch on the first HALO cols (core 0 kills its pads)
            for m in range(6):
                nc.vector.tensor_scalar(gi_sb[:, m, :HALO], gi_sb[:, m, :HALO],
                                        patch_sb[:, m:m + 1],
                                        patch_sb[:, 6 + m:7 + m],
                                        ALU.mult, ALU.add)

            # ---- GRU fixed-point sweeps (Jacobi, ping-pong h buffers) ----
            hsh = [big.tile([KP, 2, L + 1], f16, name=f"hsh{i}") for i in range(2)]
            for i in range(2):
                for mm in range(2):
                    nc.vector.memset(hsh[i][:, mm, :], 0.0)
            for s in range(SWEEPS):
                hr = hsh[s % 2]
                hw = hsh[1 - s % 2]
                z_sb = big.tile([KP, 2, L], f16, tag="Z")
                b_sb = big.tile([KP, 2, L], f16, tag="B")
                for ci, (c0, c1) in enumerate(ch2):
                    cw = c1 - c0
                    psg = [psG.tile([KP, 512], f32, tag=f"G{m}",
                                    name=f"psu_{s}_{m}") for m in range(6)]
                    # u_rz = GI (identity matmul) + W_hh_rz @ h_prev
                    for m in range(4):
                        nc.tensor.matmul(psg[m][:, :cw], ident_sb[:],
                                         gi_sb[:, m, c0:c1],
                                         start=True, stop=False)
                    for m in range(6):
                        for k in range(2):
                            nc.tensor.matmul(psg[m][:, :cw],
                                             whht_sb[:, k, m * KP:(m + 1) * KP],
                                             hr[:, k, c0:c1],
                                             start=(m >= 4 and k == 0),
                                             stop=(k == 1))
                    for mm in range(2):
                        r_t = tpool.tile([KP, 512], f16, tag="r")
                        t_t = tpool.tile([KP, 512], f16, tag="t")
                        un_t = tpool.tile([KP, 512], f16, tag="un")
                        n_t = tpool.tile([KP, 512], f16, tag="n")
                        nc.scalar.activation(r_t[:, :cw], psg[mm][:, :cw],
                                             AF.Sigmoid)
                        nc.scalar.activation(z_sb[:, mm, c0:c1],
                                             psg[2 + mm][:, :cw], AF.Sigmoid)
                        # t = (gh_n + b_hh_n) * r  in one DVE op off PSUM
                        nc.vector.scalar_tensor_tensor(
                            t_t[:, :cw], psg[4 + mm][:, :cw],
                            bhn_sb[:, mm:mm + 1], r_t[:, :cw],
                            ALU.add, ALU.mult)
                        nc.gpsimd.tensor_add(un_t[:, :cw], t_t[:, :cw],
                                             gi_sb[:, 4 + mm, c0:c1])
                        nc.scalar.activation(n_t[:, :cw], un_t[:, :cw], AF.Tanh)
                        # b = (z-1)*n; scan uses op1=subtract so
                        # h = z*h_prev - b = z*h_prev + (1-z)*n
                        nc.vector.scalar_tensor_tensor(
                            b_sb[:, mm, c0:c1], z_sb[:, mm, c0:c1], 1.0,
                            n_t[:, :cw], ALU.subtract, ALU.mult)
                    # chunk-chained exact scans; overlap next chunk's gates
                    for mm in range(2):
                        nc.vector.tensor_tensor_scan(
                            hw[:, mm, c0 + 1:c1 + 1], z_sb[:, mm, c0:c1],
                            b_sb[:, mm, c0:c1], hw[:, mm, c0:c0 + 1],
                            ALU.mult, ALU.subtract)

            # ---- final Linear on the real rows (skip halo) ----
            hfin = hsh[SWEEPS % 2]
            out_sb = cpool.tile([4, ROWS], f32)
            for c0, c1 in ch1:
                cw = c1 - c0
                psf = psxw.tile([KP, 512], f32, tag="xwps")
                for k in range(2):
                    nc.tensor.matmul(psf[:OUT, :cw], fcwt_sb[:, k, :],
                                     hfin[:, k, HALO + 1 + c0:HALO + 1 + c1],
                                     start=(k == 0), stop=(k == 1))
                nc.scalar.activation(out_sb[:OUT, c0:c1], psf[:OUT, :cw],
                                     AF.Identity, bias=fcb_sb[:OUT, :])
            nc.sync.dma_start(out_d[:], out_sb[:OUT, :])

            psG_cm.__exit__(None, None, None)
            xe_cm.__exit__(None, None, None)

    nc.compile()
    return nc


def host_prepare(inputs):
    """Build the per-core input maps from the full problem inputs."""
    import ml_dtypes

    f8 = ml_dtypes.float8_e4m3
    x = np.asarray(inputs["x"], np.float32)
    ei = np.asarray(inputs["edge_index"])
    W1 = np.asarray(inputs["W1"], np.float32)
    b1 = np.asarray(inputs["b1"], np.float32)
    W2 = np.asarray(inputs["W2"], np.float32)
    b2 = np.asarray(inputs["b2"], np.float32)
    W_ih = np.asarray(inputs["W_ih"], np.float32)
    W_hh = np.asarray(inputs["W_hh"], np.float32)
    b_ih = np.asarray(inputs["b_ih"], np.float32)
    b_hh = np.asarray(inputs["b_hh"], np.float32)
    fc_w = np.asarray(inputs["fc_w"], np.float32)
    fc_b = np.asarray(inputs["fc_b"], np.float32)

    N = NUM_NODES
    src, dst = ei[0].astype(np.int64), ei[1].astype(np.int64)
    deg = np.bincount(dst, minlength=N).astype(np.float64) + 1.0
    dinv = (1.0 / np.sqrt(deg)).astype(np.float32)

    # Exact integer adjacency counts (A + I), transposed view A8[s, d]
    A8 = np.zeros((N, N), np.int8)
    np.add.at(A8, (src, dst), 1)
    idx = np.arange(N)
    A8[idx, idx] += 1

    # node enumeration for the 80 global K-tiles: tile (c,j), partition p
    # -> node c*1250 + j*128 + p (invalid slots padded)
    enum = np.full(NT * KP, -1, np.int64)
    for c in range(CORES):
        for j in range(NJ):
            base = c * ROWS + j * KP
            cnt = min(KP, ROWS - j * KP)
            s0 = (c * NJ + j) * KP
            enum[s0:s0 + cnt] = base + np.arange(cnt)
    valid = enum >= 0
    env = enum[valid]

    # x prescaled by dinv, laid out [128 part, tile, feat]
    xd = (x * dinv[:, None]).astype(np.float32)
    xe_g = np.zeros((NT * KP, IN_FEAT), np.float32)
    xe_g[valid] = xd[env]

    common = {
        "w1": W1.astype(np.float16),
        "w2": W2.astype(np.float16),
        "wiht": W_ih.T.astype(np.float16),
        "whht": W_hh.T.astype(np.float16),
        "fcwt": fc_w.T.astype(np.float16),
        "ident": np.eye(KP, dtype=np.float16),
        "b1c": b1.reshape(2, KP).T.astype(np.float32).copy(),
        "b2c": b2.reshape(2, KP).T.astype(np.float32).copy(),
        "gib": (b_ih + np.concatenate([b_hh[:2 * HID],
                                       np.zeros(HID, np.float32)])
                ).reshape(6, KP).T.astype(np.float32).copy(),
        "bhn": b_hh[2 * HID:].reshape(2, KP).T.astype(np.float32).copy(),
        "fcb": np.concatenate([fc_b, np.zeros(KP - OUT, np.float32)]
                              ).reshape(KP, 1),
    }

    in_maps = []
    for c in range(CORES):
        r0, r1 = c * ROWS, (c + 1) * ROWS
        lo = r0 - HALO
        # per-core strip of adjacency columns [lo, r1), rows in K-enum order
        strip = np.zeros((NT2 * KP, L), np.int8)
        if c == 0:
            strip[:NT * KP][valid, HALO:] = A8[env, 0:r1]
        else:
            strip[:NT * KP][valid, :] = A8[env, lo:r1]
        # duplicate own tiles at the end; zero them in the global block
        o0, o1 = c * NJ * KP, (c + 1) * NJ * KP
        strip[NT * KP:] = strip[o0:o1]
        strip[o0:o1] = 0
        # interleave in groups of GK tiles: row g*128+p, col j5*L+cc
        adj = np.ascontiguousarray(
            strip.reshape(NG2, GK, KP, L).transpose(0, 2, 1, 3)
        ).reshape(NG2 * KP, GK * L).astype(f8)

        xe_e = np.zeros((NT2 * KP, IN_FEAT), np.float32)
        xe_e[:NT * KP] = xe_g
        xe_e[NT * KP:] = xe_g[o0:o1]
        xe = np.ascontiguousarray(
            xe_e.reshape(NT2, KP, IN_FEAT).transpose(1, 0, 2)
        ).reshape(KP, NT2 * IN_FEAT).astype(np.float16)

        # dinv of the strip's column nodes, broadcast over partitions
        drow = np.zeros(L, np.float32)
        if c == 0:
            drow[HALO:] = dinv[0:r1]
        else:
            drow[:] = dinv[lo:r1]
        dr = np.broadcast_to(drow.astype(np.float16), (KP, L)).copy()

        # dinv per (partition, local tile) for XW2 row scaling (0 on pads)
        dv = np.zeros((KP, NJ), np.float32)
        for j in range(NJ):
            cnt = min(KP, ROWS - j * KP)
            dv[:cnt, j] = dinv[r0 + j * KP:r0 + j * KP + cnt]

        patch = np.zeros((KP, 12), np.float32)
        if c == 0:
            # mul=0; add=-60 for r,z gate tiles, 0 for n tiles -> pad cols
            # produce exactly h=0 so row 0 starts from the true h0=0.
            patch[:, 6:10] = -60.0
        else:
            patch[:, 0:6] = 1.0
        in_maps.append({**common, "adj": adj, "xe": xe, "dr": dr, "dv": dv,
                        "patch": patch})
    return in_maps


def assemble_output(results):
    outs = [r["out_t"].T for r in results]          # each [ROWS, OUT]
    full = np.concatenate(outs, axis=0).astype(np.float32)
    return full[None]                               # [1, N, OUT]


def kernel(**inputs) -> np.ndarray:
    from concourse import bass_utils

    if "nc" not in _CACHE:
        _CACHE["nc"] = build_program()
    nc = _CACHE["nc"]
    in_maps = host_prepare(inputs)
    res = bass_utils.run_bass_kernel_spmd(
        nc, in_maps, core_ids=list(range(CORES)))
    return assemble_output(res.results)


if __name__ == "__main__":
    import reference

    inputs = {k: np.asarray(v) for k, v in reference.setup_inputs().items()}
    out = kernel(**inputs)
    print("kernel out", out.shape, out.dtype)
    np.save("/root/problem/kernel_out.npy", out)


# revision 23
# speedup vs baseline: 1.0127x; 1.0073x over previous
"""DCRNN (2x GCNConv + GRU-over-nodes + Linear) on 8 Trainium2 cores.

Strategy (v2)
-------------
* Adjacency is stored as EXACT small-integer edge counts (A+I) in fp8e3
  (E3M4); the D^-1/2 normalization is factored out: host prescales x
  rows by dinv, the device prescales XW2 rows (per-partition scalar) and
  output columns (broadcast dinv row).  Mixed-dtype matmul (fp16
  stationary x fp8 moving) is exact on HW, and fp8 halves adjacency HBM
  traffic vs fp16.
* GCN1 is computed as (A @ x) @ W1 (x is only 64 features wide), so the
  big A-pass runs with M=64: one matmul per K-tile instead of two.
* K-enumeration: 90 tiles of 128 rows: 80 "global" (core, j) tiles with
  each core's own tiles zeroed, plus that core's 10 tiles duplicated at
  the end.  Both GCN layers stream one adjacency layout (interleaved in
  groups of 5 tiles -> 6.5KB DMA descriptors); GCN2 starts on the own
  tiles (local XW2 shard + adjacency groups retained in SBUF from GCN1)
  while the XW2 AllGather is still in flight.
* The XW2 AllGather uses a tiled layout ([128 part, 10*256] per core,
  5KB descriptors) and is split in two (tiles j<5 / j>=5) so GCN2's
  even groups only wait on the first half.
* GRU over the 10000-node sequence: 8 Jacobi fixed-point sweeps; gates
  from the previous sweep's h (ping-pong buffers), then the recurrence
  h_t = z_t h_{t-1} + (1-z_t) n_t applied EXACTLY with the DVE affine
  scan, chunk-chained (per-partition initial state) so scans overlap
  the next chunk's gate matmuls.  Pointwise work is spread over
  scalar/vector/gpsimd.  A 64-row halo makes cores independent.
"""

import numpy as np

NUM_NODES = 10000
IN_FEAT = 64
HID = 256
OUT = 3
CORES = 8
ROWS = NUM_NODES // CORES          # 1250
HALO = 64
L = ROWS + HALO                    # 1314 local sequence length
SWEEPS = 8
KP = 128
NJ = 10                            # local node tiles per core (10*128 >= 1250)
NT = CORES * NJ                    # 80 global K-tiles
NT2 = NT + NJ                      # + own tiles duplicated at the end
GK = 5                             # K-tiles per interleave group
NG2 = NT2 // GK                    # 18 groups; own tiles = groups 16,17

_CACHE = {}


def _chunks(total, step=512):
    return [(c, min(c + step, total)) for c in range(0, total, step)]


def build_program():
    import concourse.mybir as mybir
    import concourse.tile as tile
    from concourse import bacc

    f16 = mybir.dt.float16
    f8 = mybir.dt.float8e4
    f32 = mybir.dt.float32
    DR = mybir.MatmulPerfMode.DoubleRow
    AF = mybir.ActivationFunctionType
    ALU = mybir.AluOpType

    nc = bacc.Bacc("TRN2", num_devices=CORES)

    # ---- inputs ----
    adj_d = nc.dram_tensor("adj", [NG2 * KP, GK * L], f8, kind="ExternalInput")
    xe_d = nc.dram_tensor("xe", [KP, NT2 * IN_FEAT], f16, kind="ExternalInput")
    w1_d = nc.dram_tensor("w1", [IN_FEAT, HID], f16, kind="ExternalInput")
    w2_d = nc.dram_tensor("w2", [HID, HID], f16, kind="ExternalInput")
    wiht_d = nc.dram_tensor("wiht", [HID, 3 * HID], f16, kind="ExternalInput")
    whht_d = nc.dram_tensor("whht", [HID, 3 * HID], f16, kind="ExternalInput")
    fcwt_d = nc.dram_tensor("fcwt", [HID, OUT], f16, kind="ExternalInput")
    ident_d = nc.dram_tensor("ident", [KP, KP], f16, kind="ExternalInput")
    dr_d = nc.dram_tensor("dr", [KP, L], f16, kind="ExternalInput")
    dv_d = nc.dram_tensor("dv", [KP, NJ], f32, kind="ExternalInput")
    b1c_d = nc.dram_tensor("b1c", [KP, 2], f32, kind="ExternalInput")
    b2c_d = nc.dram_tensor("b2c", [KP, 2], f32, kind="ExternalInput")
    gib_d = nc.dram_tensor("gib", [KP, 6], f32, kind="ExternalInput")
    bhn_d = nc.dram_tensor("bhn", [KP, 2], f32, kind="ExternalInput")
    fcb_d = nc.dram_tensor("fcb", [KP, 1], f32, kind="ExternalInput")
    patch_d = nc.dram_tensor("patch", [KP, 12], f32, kind="ExternalInput")
    out_d = nc.dram_tensor("out_t", [OUT, ROWS], f32, kind="ExternalOutput")

    ch1 = _chunks(ROWS)            # GCN1 output cols (own rows only)
    ch2 = _chunks(L)               # GCN2 / GRU cols (with halo)

    with tile.TileContext(nc) as tc:
        with (
            tc.tile_pool(name="const", bufs=1) as cpool,
            tc.tile_pool(name="big", bufs=1) as big,
            tc.tile_pool(name="own", bufs=1) as ownp,
            tc.tile_pool(name="tmp", bufs=4) as tpool,
            tc.tile_pool(name="psxw", bufs=2, space="PSUM") as psxw,
            tc.tile_pool(name="dram", bufs=1, space="DRAM") as dpool,
        ):
            # ---- load constants ----
            xe_cm = tc.tile_pool(name="xep", bufs=1)
            xep = xe_cm.__enter__()
            xe_sb = xep.tile([KP, NT2 * IN_FEAT], f16)
            w1_sb = cpool.tile([IN_FEAT, HID], f16)
            w2_sb = cpool.tile([KP, 2, HID], f16)
            wiht_sb = cpool.tile([KP, 2, 3 * HID], f16)
            whht_sb = cpool.tile([KP, 2, 3 * HID], f16)
            fcwt_sb = cpool.tile([KP, 2, OUT], f16)
            ident_sb = cpool.tile([KP, KP], f16)
            dr_sb = cpool.tile([KP, L], f16)
            dv_sb = cpool.tile([KP, NJ], f32)
            b1c_sb = cpool.tile([KP, 2], f32)
            b2c_sb = cpool.tile([KP, 2], f32)
            gib_sb = cpool.tile([KP, 6], f32)
            bhn_sb = cpool.tile([KP, 2], f32)
            fcb_sb = cpool.tile([KP, 1], f32)
            patch_sb = cpool.tile([KP, 12], f32)

            # adjacency groups + xe stream on sync/scalar; everything else
            # (needed later) loads via the otherwise-idle gpsimd/vector queues
            nc.sync.dma_start(ident_sb[:], ident_d[:])
            nc.gpsimd.dma_start(w1_sb[:], w1_d[:])
            hx = NT2 * IN_FEAT // 2
            nc.gpsimd.dma_start(xe_sb[:, 0:hx], xe_d[:, 0:hx])
            nc.gpsimd.dma_start(xe_sb[:, hx:], xe_d[:, hx:])
            nc.gpsimd.dma_start(dr_sb[:], dr_d[:])
            nc.gpsimd.dma_start(dv_sb[:], dv_d[:])
            for k in range(2):
                nc.gpsimd.dma_start(w2_sb[:, k, :], w2_d[k * KP:(k + 1) * KP, :])
                nc.gpsimd.dma_start(wiht_sb[:, k, :], wiht_d[k * KP:(k + 1) * KP, :])
                nc.gpsimd.dma_start(whht_sb[:, k, :], whht_d[k * KP:(k + 1) * KP, :])
                nc.gpsimd.dma_start(fcwt_sb[:, k, :], fcwt_d[k * KP:(k + 1) * KP, :])
            nc.gpsimd.dma_start(b1c_sb[:], b1c_d[:])
            nc.gpsimd.dma_start(b2c_sb[:], b2c_d[:])
            nc.gpsimd.dma_start(gib_sb[:], gib_d[:])
            nc.gpsimd.dma_start(bhn_sb[:], bhn_d[:])
            nc.gpsimd.dma_start(fcb_sb[:], fcb_d[:])
            nc.gpsimd.dma_start(patch_sb[:], patch_d[:])

            psG_cm = tc.tile_pool(name="psG", bufs=1, space="PSUM")
            psG = psG_cm.__enter__()

            # tiny AllGather to absorb the first-collective ncfw setup cost
            ccw_in = dpool.tile([CORES, 64], f16)
            ccw_out = dpool.tile([CORES * CORES, 64], f16, addr_space="Shared")
            nc.sync.dma_start(ccw_in[0:8, :], ident_sb[0:8, 0:64])
            nc.gpsimd.collective_compute(
                "AllGather", mybir.AluOpType.bypass,
                replica_groups=[list(range(CORES))],
                ins=[ccw_in.opt()], outs=[ccw_out.opt()])

            # PE warm-up burst so the HAM clock-gate opens before GCN1
            for i in range(40):
                psd = psxw.tile([KP, 512], f32, tag="xwps", name=f"warm_{i}")
                nc.tensor.matmul(psd[:, :KP], ident_sb[:], ident_sb[:],
                                 start=True, stop=True)

            ap_cm = tc.tile_pool(name="astream", bufs=6)
            apool = ap_cm.__enter__()

            # ---- GCN1 A-pass: AxT[f, col] = sum_n x_scaled[n, f] A[n, col] ----
            # K-loop over 18 groups; own groups (16, 17) last, retained for GCN2.
            psA = [psG.tile([KP, 512], f32, tag=f"G{ci}", name=f"psA_{ci}")
                   for ci in range(3)]
            own_at = {}
            for g in range(NG2):
                if g >= NG2 - 2:
                    at = ownp.tile([KP, GK, L], f8, name=f"own_{g}")
                    own_at[g] = at
                else:
                    at = apool.tile([KP, GK, L], f8, tag="a")
                eng = nc.sync if g % 2 == 0 else nc.scalar
                eng.dma_start(at[:], adj_d[g * KP:(g + 1) * KP, :])
                for j5 in range(GK):
                    t = g * GK + j5
                    for ci, (c0, c1) in enumerate(ch1):
                        nc.tensor.matmul(
                            psA[ci][:IN_FEAT, :c1 - c0],
                            xe_sb[:, t * IN_FEAT:(t + 1) * IN_FEAT],
                            at[:, j5, HALO + c0:HALO + c1],
                            start=(t == 0), stop=(t == NT2 - 1))

            # ---- h1T = relu(W1^T @ (dinv_col * AxT) + b1), own cols only ----
            ax_sb = big.tile([IN_FEAT, ROWS], f16)
            for ci, (c0, c1) in enumerate(ch1):
                nc.vector.tensor_mul(ax_sb[:, c0:c1], psA[ci][:IN_FEAT, :c1 - c0],
                                     dr_sb[0:IN_FEAT, HALO + c0:HALO + c1])
            h1t_sb = big.tile([KP, 2, NJ * KP], f16)
            for mm in range(2):
                nc.vector.memset(h1t_sb[:, mm, ROWS:], 0.0)
            for ci, (c0, c1) in enumerate(ch1):
                for mm in range(2):
                    ps = psxw.tile([KP, 512], f32, tag="xwps")
                    nc.tensor.matmul(ps[:, :c1 - c0],
                                     w1_sb[:, mm * KP:(mm + 1) * KP],
                                     ax_sb[:, c0:c1], start=True, stop=True)
                    nc.scalar.activation(h1t_sb[:, mm, c0:c1], ps[:, :c1 - c0],
                                         AF.Relu, bias=b1c_sb[:, mm:mm + 1])

            # ---- XW2 shard (natural layout), row-scaled by dinv; AllGather.
            # (fp16: an fp8e4 xw2 + DoubleRow GCN2 was 1.7x faster on the PE
            # but cost 1.3e-2 of output error -- too close to the 2e-2 gate.)
            xw2l_sb = cpool.tile([KP, NJ, HID], f16)
            bounce_a = dpool.tile([KP, GK * HID], f16)
            bounce_b = dpool.tile([KP, GK * HID], f16)
            gath_a = dpool.tile([CORES * KP, GK * HID], f16, addr_space="Shared")
            gath_b = dpool.tile([CORES * KP, GK * HID], f16, addr_space="Shared")
            for j in range(NJ):
                ps = psxw.tile([KP, 512], f32, tag="xwps")
                for k in range(2):
                    nc.tensor.matmul(ps[:, :HID],
                                     h1t_sb[:, k, j * KP:(j + 1) * KP],
                                     w2_sb[:, k, :],
                                     start=(k == 0), stop=(k == 1))
                nc.scalar.activation(xw2l_sb[:, j, :], ps[:, :HID], AF.Copy,
                                     scale=dv_sb[:, j:j + 1])
                if j == GK - 1:
                    nc.sync.dma_start(bounce_a[:], xw2l_sb[:, 0:GK, :])
                    nc.gpsimd.collective_compute(
                        "AllGather", mybir.AluOpType.bypass,
                        replica_groups=[list(range(CORES))],
                        ins=[bounce_a.opt()], outs=[gath_a.opt()])
            nc.sync.dma_start(bounce_b[:], xw2l_sb[:, GK:NJ, :])
            nc.gpsimd.collective_compute(
                "AllGather", mybir.AluOpType.bypass,
                replica_groups=[list(range(CORES))],
                ins=[bounce_b.opt()], outs=[gath_b.opt()])
            xw2g_sb = big.tile([KP, CORES, NJ, HID], f16)
            for c in range(CORES):
                eng = nc.sync if c % 2 == 0 else nc.gpsimd
                eng.dma_start(xw2g_sb[:, c, 0:GK, :],
                              gath_a[c * KP:(c + 1) * KP, :])
            for c in range(CORES):
                eng = nc.scalar if c % 2 == 0 else nc.gpsimd
                eng.dma_start(xw2g_sb[:, c, GK:NJ, :],
                              gath_b[c * KP:(c + 1) * KP, :])

            # ---- GCN2 over the extended (halo) strip ----
            # K-order: own duplicated tiles first (local XW2 + retained
            # adjacency -> runs during the AllGather), then even groups
            # (gather A), then odd groups (gather B).
            ps2 = [[psG.tile([KP, 512], f32, tag=f"G{mm * 3 + ci}",
                             name=f"ps2_{mm}_{ci}")
                    for ci in range(3)] for mm in range(2)]

            def gcn2_one(at, j5, lhs_of, first, last):
                for mm in range(2):
                    lhsT = lhs_of(mm)
                    for ci, (c0, c1) in enumerate(ch2):
                        nc.tensor.matmul(
                            ps2[mm][ci][:, :c1 - c0], lhsT,
                            at[:, j5, c0:c1],
                            start=first, stop=last)

            for g in (NG2 - 2, NG2 - 1):
                for j5 in range(GK):
                    j = (g - (NG2 - 2)) * GK + j5
                    gcn2_one(own_at[g], j5,
                             lambda mm, j=j: xw2l_sb[:, j, mm * KP:(mm + 1) * KP],
                             g == NG2 - 2 and j5 == 0, False)
            glist = [x for x in range(NT // NJ * 2) if x % 2 == 0] + \
                    [x for x in range(NT // NJ * 2) if x % 2 == 1]
            for gi_, g in enumerate(glist):
                at = apool.tile([KP, GK, L], f8, tag="a")
                eng = nc.sync if g % 2 == 0 else nc.scalar
                eng.dma_start(at[:], adj_d[g * KP:(g + 1) * KP, :])
                cc = g // 2
                for j5 in range(GK):
                    j = (g % 2) * GK + j5
                    gcn2_one(at, j5,
                             lambda mm, cc=cc, j=j: xw2g_sb[
                                 :, cc, j, mm * KP:(mm + 1) * KP],
                             False, gi_ == len(glist) - 1 and j5 == GK - 1)

            # h2 = relu(dinv_col * agg + b2)
            h2t_sb = big.tile([KP, 2, L], f16)
            for mm in range(2):
                for ci, (c0, c1) in enumerate(ch2):
                    tt = tpool.tile([KP, 512], f16, tag="h2tmp")
                    nc.vector.tensor_mul(tt[:, :c1 - c0],
                                         ps2[mm][ci][:, :c1 - c0],
                                         dr_sb[:, c0:c1])
                    nc.scalar.activation(h2t_sb[:, mm, c0:c1], tt[:, :c1 - c0],
                                         AF.Relu, bias=b2c_sb[:, mm:mm + 1])

            ap_cm.__exit__(None, None, None)

            # ---- GI = W_ih @ h2T + (b_ih [+ b_hh for r,z]) ----
            gi_sb = big.tile([KP, 6, L], f16)
            for ci, (c0, c1) in enumerate(ch2):
                cw = c1 - c0
                psg = [psG.tile([KP, 512], f32, tag=f"G{m}", name=f"psgi_{m}")
                       for m in range(6)]
                for m in range(6):
                    for k in range(2):
                        nc.tensor.matmul(psg[m][:, :cw],
                                         wiht_sb[:, k, m * KP:(m + 1) * KP],
                                         h2t_sb[:, k, c0:c1],
                                         start=(k == 0), stop=(k == 1))
                for m in range(6):
                    if m % 2 == 0:
                        nc.scalar.activation(gi_sb[:, m, c0:c1], psg[m][:, :cw],
                                             AF.Identity, bias=gib_sb[:, m:m + 1])
                    else:
                        nc.vector.tensor_scalar_add(gi_sb[:, m, c0:c1],
                                                    psg[m][:, :cw],
                                                    gib_sb[:, m:m + 1])
            # per-core GI patch on the first HALO cols (core 0 kills its pads)
            for m in range(6):
                nc.vector.tensor_scalar(gi_sb[:, m, :HALO], gi_sb[:, m, :HALO],
                                        patch_sb[:, m:m + 1],
                                        patch_sb[:, 6 + m:7 + m],
                                        ALU.mult, ALU.add)

            # ---- GRU fixed-point sweeps (Jacobi, ping-pong h buffers) ----
            hsh = [big.tile([KP, 2, L + 1], f16, name=f"hsh{i}") for i in range(2)]
            for i in range(2):
                for mm in range(2):
                    nc.vector.memset(hsh[i][:, mm, :], 0.0)
            for s in range(SWEEPS):
                hr = hsh[s % 2]
                hw = hsh[1 - s % 2]
                z_sb = big.tile([KP, 2, L], f16, tag="Z")
                b_sb = big.tile([KP, 2, L], f16, tag="B")
                for ci, (c0, c1) in enumerate(ch2):
                    cw = c1 - c0
                    psg = [psG.tile([KP, 512], f32, tag=f"G{m}",
                                    name=f"psu_{s}_{m}") for m in range(6)]
                    # u_rz = GI (identity matmul) + W_hh_rz @ h_prev
                    for m in range(4):
                        nc.tensor.matmul(psg[m][:, :cw], ident_sb[:],
                                         gi_sb[:, m, c0:c1],
                                         start=True, stop=False)
                    for m in range(6):
                        for k in range(2):
                            nc.tensor.matmul(psg[m][:, :cw],
                                             whht_sb[:, k, m * KP:(m + 1) * KP],
                                             hr[:, k, c0:c1],
                                             start=(m >= 4 and k == 0),
                                             stop=(k == 1))
                    for mm in range(2):
                        r_t = tpool.tile([KP, 512], f16, tag="r")
                        t_t = tpool.tile([KP, 512], f16, tag="t")
                        un_t = tpool.tile([KP, 512], f16, tag="un")
                        n_t = tpool.tile([KP, 512], f16, tag="n")
                        nc.scalar.activation(r_t[:, :cw], psg[mm][:, :cw],
                                             AF.Sigmoid)
                        nc.scalar.activation(z_sb[:, mm, c0:c1],
                                             psg[2 + mm][:, :cw], AF.Sigmoid)
                        # t = (gh_n + b_hh_n) * r  in one DVE op off PSUM
                        nc.vector.scalar_tensor_tensor(
                            t_t[:, :cw], psg[4 + mm][:, :cw],
                            bhn_sb[:, mm:mm + 1], r_t[:, :cw],
                            ALU.add, ALU.mult)
                        nc.gpsimd.tensor_add(un_t[:, :cw], t_t[:, :cw],
                                             gi_sb[:, 4 + mm, c0:c1])
                        nc.scalar.activation(n_t[:, :cw], un_t[:, :cw], AF.Tanh)
                        # b = (z-1)*n; scan uses op1=subtract so
                        # h = z*h_prev - b = z*h_prev + (1-z)*n
                        nc.vector.scalar_tensor_tensor(
                            b_sb[:, mm, c0:c1], z_sb[:, mm, c0:c1], 1.0,
                            n_t[:, :cw], ALU.subtract, ALU.mult)
                    # chunk-chained exact scans; overlap next chunk's gates
                    for mm in range(2):
                        nc.vector.tensor_tensor_scan(
                            hw[:, mm, c0 + 1:c1 + 1], z_sb[:, mm, c0:c1],
                            b_sb[:, mm, c0:c1], hw[:, mm, c0:c0 + 1],
                            ALU.mult, ALU.subtract)

            # ---- final Linear on the real rows (skip halo) ----
            hfin = hsh[SWEEPS % 2]
            out_sb = cpool.tile([4, ROWS], f32)
            for c0, c1 in ch1:
                cw = c1 - c0
                psf = psxw.tile([KP, 512], f32, tag="xwps")
                for k in range(2):
                    nc.tensor.matmul(psf[:OUT, :cw], fcwt_sb[:, k, :],
                                     hfin[:, k, HALO + 1 + c0:HALO + 1 + c1],
                                     start=(k == 0), stop=(k == 1))
                nc.scalar.activation(out_sb[:OUT, c0:c1], psf[:OUT, :cw],
                                     AF.Identity, bias=fcb_sb[:OUT, :])
            nc.sync.dma_start(out_d[:], out_sb[:OUT, :])

            psG_cm.__exit__(None, None, None)
            xe_cm.__exit__(None, None, None)

    nc.compile()
    return nc


def host_prepare(inputs):
    """Build the per-core input maps from the full problem inputs."""
    import ml_dtypes

    f8 = ml_dtypes.float8_e4m3
    x = np.asarray(inputs["x"], np.float32)
    ei = np.asarray(inputs["edge_index"])
    W1 = np.asarray(inputs["W1"], np.float32)
    b1 = np.asarray(inputs["b1"], np.float32)
    W2 = np.asarray(inputs["W2"], np.float32)
    b2 = np.asarray(inputs["b2"], np.float32)
    W_ih = np.asarray(inputs["W_ih"], np.float32)
    W_hh = np.asarray(inputs["W_hh"], np.float32)
    b_ih = np.asarray(inputs["b_ih"], np.float32)
    b_hh = np.asarray(inputs["b_hh"], np.float32)
    fc_w = np.asarray(inputs["fc_w"], np.float32)
    fc_b = np.asarray(inputs["fc_b"], np.float32)

    N = NUM_NODES
    src, dst = ei[0].astype(np.int64), ei[1].astype(np.int64)
    deg = np.bincount(dst, minlength=N).astype(np.float64) + 1.0
    dinv = (1.0 / np.sqrt(deg)).astype(np.float32)

    # Exact integer adjacency counts (A + I), transposed view A8[s, d]
    A8 = np.zeros((N, N), np.int8)
    np.add.at(A8, (src, dst), 1)
    idx = np.arange(N)
    A8[idx, idx] += 1

    # node enumeration for the 80 global K-tiles: tile (c,j), partition p
    # -> node c*1250 + j*128 + p (invalid slots padded)
    enum = np.full(NT * KP, -1, np.int64)
    for c in range(CORES):
        for j in range(NJ):
            base = c * ROWS + j * KP
            cnt = min(KP, ROWS - j * KP)
            s0 = (c * NJ + j) * KP
            enum[s0:s0 + cnt] = base + np.arange(cnt)
    valid = enum >= 0
    env = enum[valid]

    # x prescaled by dinv, laid out [128 part, tile, feat]
    xd = (x * dinv[:, None]).astype(np.float32)
    xe_g = np.zeros((NT * KP, IN_FEAT), np.float32)
    xe_g[valid] = xd[env]

    common = {
        "w1": W1.astype(np.float16),
        "w2": W2.astype(np.float16),
        "wiht": W_ih.T.astype(np.float16),
        "whht": W_hh.T.astype(np.float16),
        "fcwt": fc_w.T.astype(np.float16),
        "ident": np.eye(KP, dtype=np.float16),
        "b1c": b1.reshape(2, KP).T.astype(np.float32).copy(),
        "b2c": b2.reshape(2, KP).T.astype(np.float32).copy(),
        "gib": (b_ih + np.concatenate([b_hh[:2 * HID],
                                       np.zeros(HID, np.float32)])
                ).reshape(6, KP).T.astype(np.float32).copy(),
        "bhn": b_hh[2 * HID:].reshape(2, KP).T.astype(np.float32).copy(),
        "fcb": np.concatenate([fc_b, np.zeros(KP - OUT, np.float32)]
                              ).reshape(KP, 1),
    }

    in_maps = []
    for c in range(CORES):
        r0, r1 = c * ROWS, (c + 1) * ROWS
        lo = r0 - HALO
        # per-core strip of adjacency columns [lo, r1), rows in K-enum order
        strip = np.zeros((NT2 * KP, L), np.int8)
        if c == 0:
            strip[:NT * KP][valid, HALO:] = A8[env, 0:r1]
        else:
            strip[:NT * KP][valid, :] = A8[env, lo:r1]
        # duplicate own tiles at the end; zero them in the global block
        o0, o1 = c * NJ * KP, (c + 1) * NJ * KP
        strip[NT * KP:] = strip[o0:o1]
        strip[o0:o1] = 0
        # interleave in groups of GK tiles: row g*128+p, col j5*L+cc
        adj = np.ascontiguousarray(
            strip.reshape(NG2, GK, KP, L).transpose(0, 2, 1, 3)
        ).reshape(NG2 * KP, GK * L).astype(f8)

        xe_e = np.zeros((NT2 * KP, IN_FEAT), np.float32)
        xe_e[:NT * KP] = xe_g
        xe_e[NT * KP:] = xe_g[o0:o1]
        xe = np.ascontiguousarray(
            xe_e.reshape(NT2, KP, IN_FEAT).transpose(1, 0, 2)
        ).reshape(KP, NT2 * IN_FEAT).astype(np.float16)

        # dinv of the strip's column nodes, broadcast over partitions
        drow = np.zeros(L, np.float32)
        if c == 0:
            drow[HALO:] = dinv[0:r1]
        else:
            drow[:] = dinv[lo:r1]
        dr = np.broadcast_to(drow.astype(np.float16), (KP, L)).copy()

        # dinv per (partition, local tile) for XW2 row scaling (0 on pads)
        dv = np.zeros((KP, NJ), np.float32)
        for j in range(NJ):
            cnt = min(KP, ROWS - j * KP)
            dv[:cnt, j] = dinv[r0 + j * KP:r0 + j * KP + cnt]

        patch = np.zeros((KP, 12), np.float32)
        if c == 0:
            # mul=0; add=-60 for r,z gate tiles, 0 for n tiles -> pad cols
            # produce exactly h=0 so row 0 starts from the true h0=0.
            patch[:, 6:10] = -60.0
        else:
            patch[:, 0:6] = 1.0
        in_maps.append({**common, "adj": adj, "xe": xe, "dr": dr, "dv": dv,
                        "patch": patch})
    return in_maps


def assemble_output(results):
    outs = [r["out_t"].T for r in results]          # each [ROWS, OUT]
    full = np.concatenate(outs, axis=0).astype(np.float32)
    return full[None]                               # [1, N, OUT]


def kernel(**inputs) -> np.ndarray:
    from concourse import bass_utils

    if "nc" not in _CACHE:
        _CACHE["nc"] = build_program()
    nc = _CACHE["nc"]
    in_maps = host_prepare(inputs)
    res = bass_utils.run_bass_kernel_spmd(
        nc, in_maps, core_ids=list(range(CORES)))
    return assemble_output(res.results)


if __name__ == "__main__":
    import reference

    inputs = {k: np.asarray(v) for k, v in reference.setup_inputs().items()}
    out = kernel(**inputs)
    print("kernel out", out.shape, out.dtype)
    np.save("/root/problem/kernel_out.npy", out)


# revision 25
# speedup vs baseline: 1.0629x; 1.0496x over previous
"""DCRNN (2x GCNConv + GRU-over-nodes + Linear) on 8 Trainium2 cores.

Strategy (v2)
-------------
* Adjacency is stored as EXACT small-integer edge counts (A+I) in fp8e3
  (E3M4); the D^-1/2 normalization is factored out: host prescales x
  rows by dinv, the device prescales XW2 rows (per-partition scalar) and
  output columns (broadcast dinv row).  Mixed-dtype matmul (fp16
  stationary x fp8 moving) is exact on HW, and fp8 halves adjacency HBM
  traffic vs fp16.
* GCN1 is computed as (A @ x) @ W1 (x is only 64 features wide), so the
  big A-pass runs with M=64: one matmul per K-tile instead of two.
* K-enumeration: 90 tiles of 128 rows: 80 "global" (core, j) tiles with
  each core's own tiles zeroed, plus that core's 10 tiles duplicated at
  the end.  Both GCN layers stream one adjacency layout (interleaved in
  groups of 5 tiles -> 6.5KB DMA descriptors); GCN2 starts on the own
  tiles (local XW2 shard + adjacency groups retained in SBUF from GCN1)
  while the XW2 AllGather is still in flight.
* The XW2 AllGather uses a tiled layout ([128 part, 10*256] per core,
  5KB descriptors) and is split in two (tiles j<5 / j>=5) so GCN2's
  even groups only wait on the first half.
* GRU over the 10000-node sequence: 8 Jacobi fixed-point sweeps; gates
  from the previous sweep's h (ping-pong buffers), then the recurrence
  h_t = z_t h_{t-1} + (1-z_t) n_t applied EXACTLY with the DVE affine
  scan, chunk-chained (per-partition initial state) so scans overlap
  the next chunk's gate matmuls.  Pointwise work is spread over
  scalar/vector/gpsimd.  A 64-row halo makes cores independent.
"""

import numpy as np

NUM_NODES = 10000
IN_FEAT = 64
HID = 256
OUT = 3
CORES = 8
ROWS = NUM_NODES // CORES          # 1250
HALO = 32
L = ROWS + HALO                    # 1314 local sequence length
SWEEPS = 8
KP = 128
NJ = 10                            # local node tiles per core (10*128 >= 1250)
NT = CORES * NJ                    # 80 global K-tiles
NT2 = NT + NJ                      # + own tiles duplicated at the end
GK = 5                             # K-tiles per interleave group
NG2 = NT2 // GK                    # 18 groups; own tiles = groups 16,17

_CACHE = {}


def _chunks(total, step=512):
    return [(c, min(c + step, total)) for c in range(0, total, step)]


def build_program():
    import concourse.mybir as mybir
    import concourse.tile as tile
    from concourse import bacc

    f16 = mybir.dt.float16
    f8 = mybir.dt.float8e4
    f32 = mybir.dt.float32
    DR = mybir.MatmulPerfMode.DoubleRow
    AF = mybir.ActivationFunctionType
    ALU = mybir.AluOpType

    nc = bacc.Bacc("TRN2", num_devices=CORES)

    # ---- inputs ----
    adj_d = nc.dram_tensor("adj", [NG2 * KP, GK * L], f8, kind="ExternalInput")
    xe_d = nc.dram_tensor("xe", [KP, NT2 * IN_FEAT], f16, kind="ExternalInput")
    w1_d = nc.dram_tensor("w1", [IN_FEAT, HID], f16, kind="ExternalInput")
    w2_d = nc.dram_tensor("w2", [HID, HID], f16, kind="ExternalInput")
    wiht_d = nc.dram_tensor("wiht", [HID, 3 * HID], f16, kind="ExternalInput")
    whht_d = nc.dram_tensor("whht", [HID, 3 * HID], f16, kind="ExternalInput")
    fcwt_d = nc.dram_tensor("fcwt", [HID, OUT], f16, kind="ExternalInput")
    ident_d = nc.dram_tensor("ident", [KP, KP], f16, kind="ExternalInput")
    dr_d = nc.dram_tensor("dr", [KP, L], f16, kind="ExternalInput")
    dv_d = nc.dram_tensor("dv", [KP, NJ], f32, kind="ExternalInput")
    b1c_d = nc.dram_tensor("b1c", [KP, 2], f32, kind="ExternalInput")
    b2c_d = nc.dram_tensor("b2c", [KP, 2], f32, kind="ExternalInput")
    gib_d = nc.dram_tensor("gib", [KP, 6], f32, kind="ExternalInput")
    bhn_d = nc.dram_tensor("bhn", [KP, 2], f32, kind="ExternalInput")
    fcb_d = nc.dram_tensor("fcb", [KP, 1], f32, kind="ExternalInput")
    patch_d = nc.dram_tensor("patch", [KP, 12], f32, kind="ExternalInput")
    out_d = nc.dram_tensor("out_t", [OUT, ROWS], f32, kind="ExternalOutput")

    ch1 = _chunks(ROWS)            # GCN1 output cols (own rows only)
    ch2 = _chunks(L)               # GCN2 / GRU cols (with halo)

    with tile.TileContext(nc) as tc:
        with (
            tc.tile_pool(name="const", bufs=1) as cpool,
            tc.tile_pool(name="big", bufs=1) as big,
            tc.tile_pool(name="own", bufs=1) as ownp,
            tc.tile_pool(name="tmp", bufs=4) as tpool,
            tc.tile_pool(name="psxw", bufs=2, space="PSUM") as psxw,
            tc.tile_pool(name="dram", bufs=1, space="DRAM") as dpool,
        ):
            # ---- load constants ----
            xe_cm = tc.tile_pool(name="xep", bufs=1)
            xep = xe_cm.__enter__()
            xe_sb = xep.tile([KP, NT2 * IN_FEAT], f16)
            w1_sb = cpool.tile([IN_FEAT, HID], f16)
            w2_sb = cpool.tile([KP, 2, HID], f16)
            wiht_sb = cpool.tile([KP, 2, 3 * HID], f16)
            whht_sb = cpool.tile([KP, 2, 3 * HID], f16)
            fcwt_sb = cpool.tile([KP, 2, OUT], f16)
            ident_sb = cpool.tile([KP, KP], f16)
            dr_sb = cpool.tile([KP, L], f16)
            dv_sb = cpool.tile([KP, NJ], f32)
            b1c_sb = cpool.tile([KP, 2], f32)
            b2c_sb = cpool.tile([KP, 2], f32)
            gib_sb = cpool.tile([KP, 6], f32)
            bhn_sb = cpool.tile([KP, 2], f32)
            fcb_sb = cpool.tile([KP, 1], f32)
            patch_sb = cpool.tile([KP, 12], f32)

            # adjacency groups + xe stream on sync/scalar; everything else
            # (needed later) loads via the otherwise-idle gpsimd/vector queues
            nc.sync.dma_start(ident_sb[:], ident_d[:])
            nc.gpsimd.dma_start(w1_sb[:], w1_d[:])
            hx = NT2 * IN_FEAT // 2
            nc.gpsimd.dma_start(xe_sb[:, 0:hx], xe_d[:, 0:hx])
            nc.gpsimd.dma_start(xe_sb[:, hx:], xe_d[:, hx:])
            nc.gpsimd.dma_start(dr_sb[:], dr_d[:])
            nc.gpsimd.dma_start(dv_sb[:], dv_d[:])
            for k in range(2):
                nc.gpsimd.dma_start(w2_sb[:, k, :], w2_d[k * KP:(k + 1) * KP, :])
                nc.gpsimd.dma_start(wiht_sb[:, k, :], wiht_d[k * KP:(k + 1) * KP, :])
                nc.gpsimd.dma_start(whht_sb[:, k, :], whht_d[k * KP:(k + 1) * KP, :])
                nc.gpsimd.dma_start(fcwt_sb[:, k, :], fcwt_d[k * KP:(k + 1) * KP, :])
            nc.gpsimd.dma_start(b1c_sb[:], b1c_d[:])
            nc.gpsimd.dma_start(b2c_sb[:], b2c_d[:])
            nc.gpsimd.dma_start(gib_sb[:], gib_d[:])
            nc.gpsimd.dma_start(bhn_sb[:], bhn_d[:])
            nc.gpsimd.dma_start(fcb_sb[:], fcb_d[:])
            nc.gpsimd.dma_start(patch_sb[:], patch_d[:])

            psG_cm = tc.tile_pool(name="psG", bufs=1, space="PSUM")
            psG = psG_cm.__enter__()

            # tiny AllGather to absorb the first-collective ncfw setup cost
            ccw_in = dpool.tile([CORES, 64], f16)
            ccw_out = dpool.tile([CORES * CORES, 64], f16, addr_space="Shared")
            nc.sync.dma_start(ccw_in[0:8, :], ident_sb[0:8, 0:64])
            nc.gpsimd.collective_compute(
                "AllGather", mybir.AluOpType.bypass,
                replica_groups=[list(range(CORES))],
                ins=[ccw_in.opt()], outs=[ccw_out.opt()])

            # PE warm-up burst so the HAM clock-gate opens before GCN1
            for i in range(28):
                psd = psxw.tile([KP, 512], f32, tag="xwps", name=f"warm_{i}")
                nc.tensor.matmul(psd[:, :KP], ident_sb[:], ident_sb[:],
                                 start=True, stop=True)

            ap_cm = tc.tile_pool(name="astream", bufs=6)
            apool = ap_cm.__enter__()

            # ---- GCN1 A-pass: AxT[f, col] = sum_n x_scaled[n, f] A[n, col] ----
            # K-loop over 18 groups; own groups (16, 17) last, retained for GCN2.
            psA = [psG.tile([KP, 512], f32, tag=f"G{ci}", name=f"psA_{ci}")
                   for ci in range(3)]
            own_at = {}
            for g in range(NG2):
                if g >= NG2 - 2:
                    at = ownp.tile([KP, GK, L], f8, name=f"own_{g}")
                    own_at[g] = at
                else:
                    at = apool.tile([KP, GK, L], f8, tag="a")
                eng = nc.sync if g % 2 == 0 else nc.scalar
                eng.dma_start(at[:], adj_d[g * KP:(g + 1) * KP, :])
                for j5 in range(GK):
                    t = g * GK + j5
                    for ci, (c0, c1) in enumerate(ch1):
                        nc.tensor.matmul(
                            psA[ci][:IN_FEAT, :c1 - c0],
                            xe_sb[:, t * IN_FEAT:(t + 1) * IN_FEAT],
                            at[:, j5, HALO + c0:HALO + c1],
                            start=(t == 0), stop=(t == NT2 - 1))

            # ---- h1T = relu(W1^T @ (dinv_col * AxT) + b1), own cols only ----
            ax_sb = big.tile([IN_FEAT, ROWS], f16)
            for ci, (c0, c1) in enumerate(ch1):
                nc.vector.tensor_mul(ax_sb[:, c0:c1], psA[ci][:IN_FEAT, :c1 - c0],
                                     dr_sb[0:IN_FEAT, HALO + c0:HALO + c1])
            h1t_sb = big.tile([KP, 2, NJ * KP], f16)
            for mm in range(2):
                nc.vector.memset(h1t_sb[:, mm, ROWS:], 0.0)
            for ci, (c0, c1) in enumerate(ch1):
                for mm in range(2):
                    ps = psxw.tile([KP, 512], f32, tag="xwps")
                    nc.tensor.matmul(ps[:, :c1 - c0],
                                     w1_sb[:, mm * KP:(mm + 1) * KP],
                                     ax_sb[:, c0:c1], start=True, stop=True)
                    nc.scalar.activation(h1t_sb[:, mm, c0:c1], ps[:, :c1 - c0],
                                         AF.Relu, bias=b1c_sb[:, mm:mm + 1])

            # ---- XW2 shard (natural layout), row-scaled by dinv; AllGather.
            # (fp16: an fp8e4 xw2 + DoubleRow GCN2 was 1.7x faster on the PE
            # but cost 1.3e-2 of output error -- too close to the 2e-2 gate.)
            xw2l_sb = cpool.tile([KP, NJ, HID], f16)
            bounce_a = dpool.tile([KP, GK * HID], f16)
            bounce_b = dpool.tile([KP, GK * HID], f16)
            gath_a = dpool.tile([CORES * KP, GK * HID], f16, addr_space="Shared")
            gath_b = dpool.tile([CORES * KP, GK * HID], f16, addr_space="Shared")
            for j in range(NJ):
                ps = psxw.tile([KP, 512], f32, tag="xwps")
                for k in range(2):
                    nc.tensor.matmul(ps[:, :HID],
                                     h1t_sb[:, k, j * KP:(j + 1) * KP],
                                     w2_sb[:, k, :],
                                     start=(k == 0), stop=(k == 1))
                nc.scalar.activation(xw2l_sb[:, j, :], ps[:, :HID], AF.Copy,
                                     scale=dv_sb[:, j:j + 1])
                if j == GK - 1:
                    nc.sync.dma_start(bounce_a[:], xw2l_sb[:, 0:GK, :])
                    nc.gpsimd.collective_compute(
                        "AllGather", mybir.AluOpType.bypass,
                        replica_groups=[list(range(CORES))],
                        ins=[bounce_a.opt()], outs=[gath_a.opt()])
            nc.sync.dma_start(bounce_b[:], xw2l_sb[:, GK:NJ, :])
            nc.gpsimd.collective_compute(
                "AllGather", mybir.AluOpType.bypass,
                replica_groups=[list(range(CORES))],
                ins=[bounce_b.opt()], outs=[gath_b.opt()])
            xw2g_sb = big.tile([KP, CORES, NJ, HID], f16)
            for c in range(CORES):
                eng = nc.sync if c % 2 == 0 else nc.gpsimd
                eng.dma_start(xw2g_sb[:, c, 0:GK, :],
                              gath_a[c * KP:(c + 1) * KP, :])
            for c in range(CORES):
                eng = nc.scalar if c % 2 == 0 else nc.gpsimd
                eng.dma_start(xw2g_sb[:, c, GK:NJ, :],
                              gath_b[c * KP:(c + 1) * KP, :])

            # ---- GCN2 over the extended (halo) strip ----
            # K-order: own duplicated tiles first (local XW2 + retained
            # adjacency -> runs during the AllGather), then even groups
            # (gather A), then odd groups (gather B).
            ps2 = [[psG.tile([KP, 512], f32, tag=f"G{mm * 3 + ci}",
                             name=f"ps2_{mm}_{ci}")
                    for ci in range(3)] for mm in range(2)]

            def gcn2_one(at, j5, lhs_of, first, last):
                for mm in range(2):
                    lhsT = lhs_of(mm)
                    for ci, (c0, c1) in enumerate(ch2):
                        nc.tensor.matmul(
                            ps2[mm][ci][:, :c1 - c0], lhsT,
                            at[:, j5, c0:c1],
                            start=first, stop=last)

            for g in (NG2 - 2, NG2 - 1):
                for j5 in range(GK):
                    j = (g - (NG2 - 2)) * GK + j5
                    gcn2_one(own_at[g], j5,
                             lambda mm, j=j: xw2l_sb[:, j, mm * KP:(mm + 1) * KP],
                             g == NG2 - 2 and j5 == 0, False)
            glist = [x for x in range(NT // NJ * 2) if x % 2 == 0] + \
                    [x for x in range(NT // NJ * 2) if x % 2 == 1]
            for gi_, g in enumerate(glist):
                at = apool.tile([KP, GK, L], f8, tag="a")
                eng = nc.sync if g % 2 == 0 else nc.scalar
                eng.dma_start(at[:], adj_d[g * KP:(g + 1) * KP, :])
                cc = g // 2
                for j5 in range(GK):
                    j = (g % 2) * GK + j5
                    gcn2_one(at, j5,
                             lambda mm, cc=cc, j=j: xw2g_sb[
                                 :, cc, j, mm * KP:(mm + 1) * KP],
                             False, gi_ == len(glist) - 1 and j5 == GK - 1)

            # h2 = relu(dinv_col * agg + b2)
            h2t_sb = big.tile([KP, 2, L], f16)
            for mm in range(2):
                for ci, (c0, c1) in enumerate(ch2):
                    tt = tpool.tile([KP, 512], f16, tag="h2tmp")
                    nc.vector.tensor_mul(tt[:, :c1 - c0],
                                         ps2[mm][ci][:, :c1 - c0],
                                         dr_sb[:, c0:c1])
                    nc.scalar.activation(h2t_sb[:, mm, c0:c1], tt[:, :c1 - c0],
                                         AF.Relu, bias=b2c_sb[:, mm:mm + 1])

            ap_cm.__exit__(None, None, None)

            # ---- GI = W_ih @ h2T + (b_ih [+ b_hh for r,z]) ----
            gi_sb = big.tile([KP, 6, L], f16)
            for ci, (c0, c1) in enumerate(ch2):
                cw = c1 - c0
                psg = [psG.tile([KP, 512], f32, tag=f"G{m}", name=f"psgi_{m}")
                       for m in range(6)]
                for m in range(6):
                    for k in range(2):
                        nc.tensor.matmul(psg[m][:, :cw],
                                         wiht_sb[:, k, m * KP:(m + 1) * KP],
                                         h2t_sb[:, k, c0:c1],
                                         start=(k == 0), stop=(k == 1))
                for m in range(6):
                    if m % 2 == 0:
                        nc.scalar.activation(gi_sb[:, m, c0:c1], psg[m][:, :cw],
                                             AF.Identity, bias=gib_sb[:, m:m + 1])
                    else:
                        nc.vector.tensor_scalar_add(gi_sb[:, m, c0:c1],
                                                    psg[m][:, :cw],
                                                    gib_sb[:, m:m + 1])
            # per-core GI patch on the first HALO cols (core 0 kills its pads)
            for m in range(6):
                nc.vector.tensor_scalar(gi_sb[:, m, :HALO], gi_sb[:, m, :HALO],
                                        patch_sb[:, m:m + 1],
                                        patch_sb[:, 6 + m:7 + m],
                                        ALU.mult, ALU.add)

            # ---- GRU fixed-point sweeps (Jacobi, ping-pong h buffers) ----
            hsh = [big.tile([KP, 2, L + 1], f16, name=f"hsh{i}") for i in range(2)]
            for i in range(2):
                for mm in range(2):
                    nc.vector.memset(hsh[i][:, mm, :], 0.0)
            for s in range(SWEEPS):
                hr = hsh[s % 2]
                hw = hsh[1 - s % 2]
                z_sb = big.tile([KP, 2, L], f16, tag="Z")
                b_sb = big.tile([KP, 2, L], f16, tag="B")
                for ci, (c0, c1) in enumerate(ch2):
                    cw = c1 - c0
                    psg = [psG.tile([KP, 512], f32, tag=f"G{m}",
                                    name=f"psu_{s}_{m}") for m in range(6)]
                    # u_rz = GI (identity matmul) + W_hh_rz @ h_prev
                    for m in range(4):
                        nc.tensor.matmul(psg[m][:, :cw], ident_sb[:],
                                         gi_sb[:, m, c0:c1],
                                         start=True, stop=False)
                    for m in range(6):
                        for k in range(2):
                            nc.tensor.matmul(psg[m][:, :cw],
                                             whht_sb[:, k, m * KP:(m + 1) * KP],
                                             hr[:, k, c0:c1],
                                             start=(m >= 4 and k == 0),
                                             stop=(k == 1))
                    for mm in range(2):
                        r_t = tpool.tile([KP, 512], f16, tag="r")
                        t_t = tpool.tile([KP, 512], f16, tag="t")
                        un_t = tpool.tile([KP, 512], f16, tag="un")
                        n_t = tpool.tile([KP, 512], f16, tag="n")
                        nc.scalar.activation(r_t[:, :cw], psg[mm][:, :cw],
                                             AF.Sigmoid)
                        nc.scalar.activation(z_sb[:, mm, c0:c1],
                                             psg[2 + mm][:, :cw], AF.Sigmoid)
                        # t = (gh_n + b_hh_n) * r  in one DVE op off PSUM
                        nc.vector.scalar_tensor_tensor(
                            t_t[:, :cw], psg[4 + mm][:, :cw],
                            bhn_sb[:, mm:mm + 1], r_t[:, :cw],
                            ALU.add, ALU.mult)
                        nc.gpsimd.tensor_add(un_t[:, :cw], t_t[:, :cw],
                                             gi_sb[:, 4 + mm, c0:c1])
                        nc.scalar.activation(n_t[:, :cw], un_t[:, :cw], AF.Tanh)
                        # b = (z-1)*n; scan uses op1=subtract so
                        # h = z*h_prev - b = z*h_prev + (1-z)*n
                        nc.vector.scalar_tensor_tensor(
                            b_sb[:, mm, c0:c1], z_sb[:, mm, c0:c1], 1.0,
                            n_t[:, :cw], ALU.subtract, ALU.mult)
                    # chunk-chained exact scans; overlap next chunk's gates
                    for mm in range(2):
                        nc.vector.tensor_tensor_scan(
                            hw[:, mm, c0 + 1:c1 + 1], z_sb[:, mm, c0:c1],
                            b_sb[:, mm, c0:c1], hw[:, mm, c0:c0 + 1],
                            ALU.mult, ALU.subtract)

            # ---- final Linear on the real rows (skip halo) ----
            hfin = hsh[SWEEPS % 2]
            out_sb = cpool.tile([4, ROWS], f32)
            for c0, c1 in ch1:
                cw = c1 - c0
                psf = psxw.tile([KP, 512], f32, tag="xwps")
                for k in range(2):
                    nc.tensor.matmul(psf[:OUT, :cw], fcwt_sb[:, k, :],
                                     hfin[:, k, HALO + 1 + c0:HALO + 1 + c1],
                                     start=(k == 0), stop=(k == 1))
                nc.scalar.activation(out_sb[:OUT, c0:c1], psf[:OUT, :cw],
                                     AF.Identity, bias=fcb_sb[:OUT, :])
            nc.sync.dma_start(out_d[:], out_sb[:OUT, :])

            psG_cm.__exit__(None, None, None)
            xe_cm.__exit__(None, None, None)

    nc.compile()
    return nc


def host_prepare(inputs):
    """Build the per-core input maps from the full problem inputs."""
    import ml_dtypes

    f8 = ml_dtypes.float8_e4m3
    x = np.asarray(inputs["x"], np.float32)
    ei = np.asarray(inputs["edge_index"])
    W1 = np.asarray(inputs["W1"], np.float32)
    b1 = np.asarray(inputs["b1"], np.float32)
    W2 = np.asarray(inputs["W2"], np.float32)
    b2 = np.asarray(inputs["b2"], np.float32)
    W_ih = np.asarray(inputs["W_ih"], np.float32)
    W_hh = np.asarray(inputs["W_hh"], np.float32)
    b_ih = np.asarray(inputs["b_ih"], np.float32)
    b_hh = np.asarray(inputs["b_hh"], np.float32)
    fc_w = np.asarray(inputs["fc_w"], np.float32)
    fc_b = np.asarray(inputs["fc_b"], np.float32)

    N = NUM_NODES
    src, dst = ei[0].astype(np.int64), ei[1].astype(np.int64)
    deg = np.bincount(dst, minlength=N).astype(np.float64) + 1.0
    dinv = (1.0 / np.sqrt(deg)).astype(np.float32)

    # Exact integer adjacency counts (A + I), transposed view A8[s, d]
    A8 = np.zeros((N, N), np.int8)
    np.add.at(A8, (src, dst), 1)
    idx = np.arange(N)
    A8[idx, idx] += 1

    # node enumeration for the 80 global K-tiles: tile (c,j), partition p
    # -> node c*1250 + j*128 + p (invalid slots padded)
    enum = np.full(NT * KP, -1, np.int64)
    for c in range(CORES):
        for j in range(NJ):
            base = c * ROWS + j * KP
            cnt = min(KP, ROWS - j * KP)
            s0 = (c * NJ + j) * KP
            enum[s0:s0 + cnt] = base + np.arange(cnt)
    valid = enum >= 0
    env = enum[valid]

    # x prescaled by dinv, laid out [128 part, tile, feat]
    xd = (x * dinv[:, None]).astype(np.float32)
    xe_g = np.zeros((NT * KP, IN_FEAT), np.float32)
    xe_g[valid] = xd[env]

    common = {
        "w1": W1.astype(np.float16),
        "w2": W2.astype(np.float16),
        "wiht": W_ih.T.astype(np.float16),
        "whht": W_hh.T.astype(np.float16),
        "fcwt": fc_w.T.astype(np.float16),
        "ident": np.eye(KP, dtype=np.float16),
        "b1c": b1.reshape(2, KP).T.astype(np.float32).copy(),
        "b2c": b2.reshape(2, KP).T.astype(np.float32).copy(),
        "gib": (b_ih + np.concatenate([b_hh[:2 * HID],
                                       np.zeros(HID, np.float32)])
                ).reshape(6, KP).T.astype(np.float32).copy(),
        "bhn": b_hh[2 * HID:].reshape(2, KP).T.astype(np.float32).copy(),
        "fcb": np.concatenate([fc_b, np.zeros(KP - OUT, np.float32)]
                              ).reshape(KP, 1),
    }

    in_maps = []
    for c in range(CORES):
        r0, r1 = c * ROWS, (c + 1) * ROWS
        lo = r0 - HALO
        # per-core strip of adjacency columns [lo, r1), rows in K-enum order
        strip = np.zeros((NT2 * KP, L), np.int8)
        if c == 0:
            strip[:NT * KP][valid, HALO:] = A8[env, 0:r1]
        else:
            strip[:NT * KP][valid, :] = A8[env, lo:r1]
        # duplicate own tiles at the end; zero them in the global block
        o0, o1 = c * NJ * KP, (c + 1) * NJ * KP
        strip[NT * KP:] = strip[o0:o1]
        strip[o0:o1] = 0
        # interleave in groups of GK tiles: row g*128+p, col j5*L+cc
        adj = np.ascontiguousarray(
            strip.reshape(NG2, GK, KP, L).transpose(0, 2, 1, 3)
        ).reshape(NG2 * KP, GK * L).astype(f8)

        xe_e = np.zeros((NT2 * KP, IN_FEAT), np.float32)
        xe_e[:NT * KP] = xe_g
        xe_e[NT * KP:] = xe_g[o0:o1]
        xe = np.ascontiguousarray(
            xe_e.reshape(NT2, KP, IN_FEAT).transpose(1, 0, 2)
        ).reshape(KP, NT2 * IN_FEAT).astype(np.float16)

        # dinv of the strip's column nodes, broadcast over partitions
        drow = np.zeros(L, np.float32)
        if c == 0:
            drow[HALO:] = dinv[0:r1]
        else:
            drow[:] = dinv[lo:r1]
        dr = np.broadcast_to(drow.astype(np.float16), (KP, L)).copy()

        # dinv per (partition, local tile) for XW2 row scaling (0 on pads)
        dv = np.zeros((KP, NJ), np.float32)
        for j in range(NJ):
            cnt = min(KP, ROWS - j * KP)
            dv[:cnt, j] = dinv[r0 + j * KP:r0 + j * KP + cnt]

        patch = np.zeros((KP, 12), np.float32)
        if c == 0:
            # mul=0; add=-60 for r,z gate tiles, 0 for n tiles -> pad cols
            # produce exactly h=0 so row 0 starts from the true h0=0.
            patch[:, 6:10] = -60.0
        else:
            patch[:, 0:6] = 1.0
        in_maps.append({**common, "adj": adj, "xe": xe, "dr": dr, "dv": dv,
                        "patch": patch})
    return in_maps


def assemble_output(results):
    outs = [r["out_t"].T for r in results]          # each [ROWS, OUT]
    full = np.concatenate(outs, axis=0).astype(np.float32)
    return full[None]                               # [1, N, OUT]


def kernel(**inputs) -> np.ndarray:
    from concourse import bass_utils

    if "nc" not in _CACHE:
        _CACHE["nc"] = build_program()
    nc = _CACHE["nc"]
    in_maps = host_prepare(inputs)
    res = bass_utils.run_bass_kernel_spmd(
        nc, in_maps, core_ids=list(range(CORES)))
    return assemble_output(res.results)


if __name__ == "__main__":
    import reference

    inputs = {k: np.asarray(v) for k, v in reference.setup_inputs().items()}
    out = kernel(**inputs)
    print("kernel out", out.shape, out.dtype)
    np.save("/root/problem/kernel_out.npy", out)
